# revision 60
# baseline (speedup 1.0000x reference)
"""Grouped-query attention (B=2, T=2048, d_model=2048, 32 Q heads / 8 KV heads)
sharded over 8 NeuronCores: batch x head-block tensor parallel.

Core c handles batch b = c//4 and head-block hb = c%4 (8 q heads = 2 kv groups).
bf16 matmul operands everywhere (fp32 PSUM accumulate); host feeds pre-transposed
bf16 inputs and sums/transposes per-core partials.

v5: fully software-pipelined A/B/C on 1-bank PSUM tiles.
  - All PSUM accumulators are [128, 512] (one bank), so projections (A),
    attention (B) and output projection (C) can be co-resident in the 8 banks
    and overlap: A(tb) interleaves with B(2tb-2, 2tb-1); C's first token half
    fills PE slack inside B(6,7)'s exp-bound stretch; C's second half is the
    tail. B is ACT(exp)-bound, A/C are pure PE, so the stagger hides most of
    the exp cost.
  - Phase B processes query blocks j of 256 rows x 2 rep-halves; per half it
    scans exactly the causal prefix of key tiles (NKT = 2j+2) in batches of
    up to 4 (one PE tiling-mode round trip per batch). Score matmuls for the
    two kv groups auto-row-tile ((0,0)/(64,0) 64x128 PE tiles, concurrent).
    V_aug carries a 64-wide ones block so AV matmuls have M=128 and og rows
    64-127 hold the softmax denominator replicated across 64 partitions
    (vectorized reciprocal, no gpsimd broadcast). Causal masking:
    multiplicative 0/1 bf16 mask on P after exp, diagonal key tiles only.
  - Inputs stream in 4 ck-groups (phase A consumes group 0 while the rest
    are in flight); Wo loads last (phase C only).
"""

import numpy as np

D_MODEL = 2048
T = 2048
B = 2
DK = 64
NREP = 4

_CACHE: dict = {}


# --------------------------------------------------------------------------
# device kernel
# --------------------------------------------------------------------------
def _build_nc(loop_n=1, unroll=False, phases="ABC", st_bufs=4, og_bufs=4,
              interleave=False, b_no_exp=False, b_no_mask=False,
              b_no_norm=False, diag_first=False, norm_copy_dve=True,
              mask_gpsimd=False, b_exp_dve=False, norm_mul_gpsimd=True):
    import concourse.bass as bass
    import concourse.mybir as mybir
    import concourse.tile as tile
    from concourse import bacc
    from concourse.masks import make_identity

    F32 = mybir.dt.float32
    BF16 = mybir.dt.bfloat16
    EXP = mybir.ActivationFunctionType.Exp
    ds = bass.ds

    nc = bacc.Bacc("TRN2", target_bir_lowering=False, debug=False)

    xT = nc.dram_tensor("xT", [128, 16 * 2048], BF16, kind="ExternalInput").ap()
    WqT = nc.dram_tensor("WqT", [128, 16 * 512], BF16, kind="ExternalInput").ap()
    WkT = nc.dram_tensor("WkT", [128, 16 * 128], BF16, kind="ExternalInput").ap()
    WvT = nc.dram_tensor("WvT", [128, 16 * 128], BF16, kind="ExternalInput").ap()
    WoT = nc.dram_tensor("WoT", [128, 4 * 2048], BF16, kind="ExternalInput").ap()
    MSK4 = nc.dram_tensor("MSK4", [128, 1024], BF16, kind="ExternalInput").ap()
    YT = nc.dram_tensor("YT", [2048, 2048], BF16, kind="ExternalOutput").ap()

    with tile.TileContext(nc) as tc:
        def loop(n, body, **kw):
            """Hardware For_i over range(n), or python-unrolled (for the
            timeline simulator, which can't resolve reg-mode branches)."""
            if unroll:
                for v in range(n):
                    body(v)
            else:
                with tc.For_i(0, n, 1, **kw) as v:
                    body(v)

        with tc.tile_pool(name="consts", bufs=1) as consts, \
             tc.tile_pool(name="wts", bufs=1) as wts, \
             tc.tile_pool(name="persist", bufs=1) as persist, \
             tc.tile_pool(name="pp", bufs=8) as ppool, \
             tc.tile_pool(name="rcp", bufs=4) as rcp, \
             tc.tile_pool(name="ytp", bufs=4) as ytp, \
             tc.tile_pool(name="mrg", bufs=6) as mrg, \
             tc.tile_pool(name="ps", bufs=1, space="PSUM") as ps:

            # ---------------- constants (outside the timing loop) ----------
            idl_f32 = consts.tile([128, 128], F32)
            make_identity(nc, idl_f32)
            idl = consts.tile([128, 128], BF16)
            nc.vector.tensor_copy(idl, idl_f32)

            x_sb = wts.tile([128, 16 * 2048], BF16)   # col = ck*2048 + tok
            wq_sb = wts.tile([128, 16 * 512], BF16)   # col = ck*512 + qout
            wk_sb = wts.tile([128, 16 * 128], BF16)   # col = ck*128 + kout
            wv_sb = wts.tile([128, 16 * 128], BF16)
            wo_sb = wts.tile([128, 4 * 2048], BF16)   # col = ic*2048 + out
            msk_sb = consts.tile([128, 1024], BF16)   # 4 x [128,256] mask blocks

            qt_sb = persist.tile([128, 8192], BF16)   # col = j*1024 + r*256 + qi
            kt_bd = persist.tile([128, 4096], BF16)   # block-diag K tiles:
            # tile (kt,g) at cols (2kt+g)*128, K_g dk on rows 64g..64g+64,
            # zeros elsewhere (so K=128 score matmuls select one group)
            vt_sb = persist.tile([128, 2048], BF16)   # [kvd, tok]
            va_sb = persist.tile([128, 4096], BF16)   # 32 x [128 tok, 64 v | 64 ones]
            otn_sb = persist.tile([128, 8192], BF16)  # col = oc*2048 + tok
            otn2_sb = persist.tile([64, 8192], BF16)  # odd-rep rows, staged up

            nc.vector.memset(kt_bd, 0.0)   # zero rows persist across reps

            # ones block of every V_aug tile (values never change)
            ones_ap = bass.AP(tensor=va_sb.tensor, offset=va_sb.offset + 64,
                              ap=[va_sb.ap[0], [128, 32], [1, 64]])
            nc.vector.memset(ones_ap, 1.0)

            def _a_evacuate(tb, grp, acc):
                if grp < 4:      # q chunk: qt_sb col = j*1024 + qc*256 + qi
                    dst = bass.AP(
                        tensor=qt_sb.tensor,
                        offset=qt_sb.offset + tb * 2048 + grp * 256,
                        ap=[qt_sb.ap[0], [1024, 2], [1, 256]])
                    nc.vector.tensor_copy(dst, acc)
                elif grp == 5:
                    nc.vector.tensor_copy(vt_sb[:, ds(tb * 512, 512)], acc)
                else:            # K -> block-diagonal tiles
                    for g in range(2):
                        base = kt_bd[64 * g:64 * (g + 1), :]
                        dst = bass.AP(
                            tensor=base.tensor,
                            offset=base.offset + (8 * tb + g) * 128,
                            ap=[base.ap[0], [256, 4], [1, 128]])
                        nc.vector.tensor_copy(
                            dst, acc[64 * g:64 * (g + 1), :])

            def _a_w(grp, ck):
                if grp < 4:
                    return wq_sb[:, ck * 512 + grp * 128:
                                 ck * 512 + (grp + 1) * 128]
                w_t = wk_sb if grp == 4 else wv_sb
                return w_t[:, ck * 128:(ck + 1) * 128]

            def _a_mm128(tb, grp, acc, ck):
                nc.tensor.matmul(
                    acc, _a_w(grp, ck),
                    x_sb[:, ds(tb * 512 + ck * 2048, 512)],
                    start=(ck == 0), stop=(ck == 15))

            def _a_vtp(tb):
                # V_aug build for this tb's 4 key tiles: transpose VT
                # 128-blocks into [tok, vdim] tiles
                for i in range(4):
                    kt = 4 * tb + i
                    vtp = ps.tile([128, 128], BF16, tag="pa", bufs=2,
                                  name=f"vtp{kt}")
                    nc.tensor.transpose(
                        vtp, vt_sb[:, kt * 128:(kt + 1) * 128], idl)
                    # vtp cols 0-63 = g0 vdims -> va tile kt; 64-127 = g1
                    # vdims -> va tile 16+kt (ones block at +64 untouched)
                    dest = bass.AP(tensor=va_sb.tensor,
                                   offset=va_sb.offset + kt * 128,
                                   ap=[va_sb.ap[0], [16 * 128, 2], [1, 64]])
                    src = bass.AP(tensor=vtp.tensor, offset=vtp.offset,
                                  ap=[vtp.ap[0], [64, 2], [1, 64]])
                    nc.vector.tensor_copy(dest, src)

            def phase_a_wide(tb):
                # pre-B: all 6 accumulation groups in flight, ck-major, so
                # the PE tracks the input DMA stream; K=128 full mode (the
                # one tiling-mode switch happens at the A->B boundary)
                tags = ["og", "og", "og", "og", "st", "st"]
                accs = [ps.tile([128, 512], F32, tag=tags[grp],
                                bufs=(og_bufs if grp < 4 else st_bufs),
                                name=f"aw{tb}_{grp}") for grp in range(6)]
                for ck in range(16):
                    for grp in range(6):
                        _a_mm128(tb, grp, accs[grp], ck)
                for grp in range(6):
                    _a_evacuate(tb, grp, accs[grp])
                for i in range(4):
                    kt = 4 * tb + i
                    vtp = ps.tile([128, 128], BF16, tag="og", bufs=og_bufs,
                                  name=f"vtp{kt}")
                    nc.tensor.transpose(
                        vtp, vt_sb[:, kt * 128:(kt + 1) * 128], idl)
                    dest = bass.AP(tensor=va_sb.tensor,
                                   offset=va_sb.offset + kt * 128,
                                   ap=[va_sb.ap[0], [16 * 128, 2], [1, 64]])
                    src2 = bass.AP(tensor=vtp.tensor, offset=vtp.offset,
                                   ap=[vtp.ap[0], [64, 2], [1, 64]])
                    nc.vector.tensor_copy(dest, src2)

            def phase_a_wide(tb):
                # pre-B: all 6 accumulation groups in flight, ck-major, so
                # the PE tracks the input DMA stream; K=128 full mode (the
                # one tiling-mode switch happens at the A->B boundary)
                tags = ["og", "og", "og", "og", "st", "st"]
                accs = [ps.tile([128, 512], F32, tag=tags[grp],
                                bufs=(og_bufs if grp < 4 else st_bufs),
                                name=f"aw{tb}_{grp}") for grp in range(6)]
                for ck in range(16):
                    for grp in range(6):
                        _a_mm128(tb, grp, accs[grp], ck)
                for grp in range(6):
                    _a_evacuate(tb, grp, accs[grp])
                for i in range(4):
                    kt = 4 * tb + i
                    vtp = ps.tile([128, 128], BF16, tag="og", bufs=og_bufs,
                                  name=f"vtp{kt}")
                    nc.tensor.transpose(
                        vtp, vt_sb[:, kt * 128:(kt + 1) * 128], idl)
                    dest = bass.AP(tensor=va_sb.tensor,
                                   offset=va_sb.offset + kt * 128,
                                   ap=[va_sb.ap[0], [16 * 128, 2], [1, 64]])
                    src2 = bass.AP(tensor=vtp.tensor, offset=vtp.offset,
                                   ap=[vtp.ap[0], [64, 2], [1, 64]])
                    nc.vector.tensor_copy(dest, src2)

            def a_chunks(tb):
                """Phase A(tb) as background chunks: 6 accumulation groups
                (16 matmul pairs + merge-evac each) plus the V_aug
                transposes. Emitted one per phase-B batch slot so A's PE
                work fills B's exp-bound slack."""
                def grp_chunk(grp):
                    def run():
                        acc2 = [ps.tile([128, 512], F32, tag="pa", bufs=2,
                                        name=f"pa{tb}_{grp}_{hh}")
                                for hh in range(2)]
                        for ck in range(16):
                            _a_mm64(tb, grp, acc2, ck)
                        tmp = mrg.tile([128, 512], F32, tag="mrg",
                                       name=f"mt{tb}_{grp}")
                        nc.scalar.copy(tmp, acc2[1])
                        nc.vector.tensor_add(
                            qt_sb[:, 0:1], acc2[0], tmp)  # unused path
                    return run
                return ([("A", tb, grp_chunk(g)) for g in range(6)]
                        + [("A", tb, lambda: _a_vtp(tb))])

            def phase_a0():
                # ck-major variant for tb=0: all 6 accumulation groups in
                # flight (B hasn't started; its st/og banks are free), so the
                # PE tracks the input DMA stream instead of stalling on two
                # serialized groups.
                tags = ["pa", "pa", "st", "st", "og", "og"]
                tbufs = {"pa": 2, "st": st_bufs, "og": og_bufs}
                accs = [ps.tile([128, 512], F32, tag=tags[grp],
                                bufs=tbufs[tags[grp]],
                                name=f"pa0_{grp}") for grp in range(6)]
                for ck in range(16):
                    for grp in range(6):
                        _a_mm128(0, grp, accs[grp], ck)
                for grp in range(6):
                    _a_evacuate(tb, grp, accs[grp])
                for i in range(4):
                    vtp = ps.tile([128, 128], BF16, tag="pa", bufs=2,
                                  name=f"vtp0_{i}")
                    nc.tensor.transpose(
                        vtp, vt_sb[:, i * 128:(i + 1) * 128], idl)
                    dest = bass.AP(tensor=va_sb.tensor,
                                   offset=va_sb.offset + i * 128,
                                   ap=[va_sb.ap[0], [16 * 128, 2], [1, 64]])
                    src = bass.AP(tensor=vtp.tensor, offset=vtp.offset,
                                  ap=[vtp.ap[0], [64, 2], [1, 64]])
                    nc.vector.tensor_copy(dest, src)

            def _batches(nkt):
                """Key-tile batches of up to 4, DIAGONAL TILES FIRST: their
                mask multiplies then overlap later tiles' scores/exps
                instead of serializing the block tail (AV accumulation
                order is arbitrary)."""
                order = ([nkt - 2, nkt - 1] + list(range(nkt - 2))
                         if diag_first else list(range(nkt)))
                return [order[i:i + 4] for i in range(0, nkt, 4)]

            carry = [None]   # deferred tail (last AVs + normalize) per (j,hf)

            def flush_b(filler=None):
                if carry[0] is not None:
                    carry[0]()
                    carry[0] = None
                    if filler is not None:
                        filler()

            def phase_b(j, filler=None):
                """Everything in here runs in the PE's 64x128 tiling mode:
                scores are (0,0)/(64,0) row-tile pairs, and AV matmuls split
                their 128-token contraction into two 64-row halves
                accumulating into separate og banks (merged by the DVE in
                the normalize step). No tiling-mode drains in the loop.

                Each (j,hf)'s last AV batch + normalize is deferred and
                emitted inside the NEXT (j,hf)'s first batch, right after
                its first score pair, so the PE never sits at a boundary."""
                NKT = 2 * j + 2
                for hf in range(2):     # rep-halves of the 1024-wide q block
                    og = [ps.tile([128, 512], F32, tag="og", bufs=og_bufs,
                                  name=f"og{j}_{hf}_{g}") for g in range(2)]

                    def av1(pkt, g, p, first, last, og=og):
                        nc.tensor.matmul(
                            og[g],
                            va_sb[:, (g * 16 + pkt) * 128:
                                  (g * 16 + pkt) * 128 + 128],
                            p, start=first, stop=last)

                    def norm(og=og, j=j, hf=hf):
                        if b_no_norm:
                            return
                        # og rows 64-127 hold the denominator replicated
                        # across 64 partitions; recip and the normalize
                        # muls read og directly (one PSUM input each). og
                        # is double-buffered across (j,hf) blocks, so this
                        # never stalls the next block's AV matmuls.
                        for g in range(2):
                            rec = rcp.tile([64, 512], F32, tag="rec",
                                           name=f"r{j}{hf}{g}")
                            nc.vector.reciprocal(rec, og[g][64:128, :])
                            oc = 2 * g + hf
                            for r in range(2):   # rep 2hf, 2hf+1
                                dst = otn_sb if r == 0 else otn2_sb
                                nc.vector.tensor_mul(
                                    dst[0:64, ds(oc * 2048 + j * 256, 256)],
                                    og[g][0:64, r * 256:(r + 1) * 256],
                                    rec[:, r * 256:(r + 1) * 256])

                    def score(kt, g):
                        # K=128 full-mode matmul; the block-diagonal zero
                        # rows of the weight tile select group g from the
                        # stacked Q (no tiling-mode changes anywhere)
                        st = ps.tile([128, 512], F32, tag="st",
                                     bufs=st_bufs, name=f"st{j}{hf}{kt}{g}")
                        nc.tensor.matmul(
                            st,
                            kt_bd[:, (2 * kt + g) * 128:
                                  (2 * kt + g + 1) * 128],
                            qt_sb[:, ds(j * 1024 + hf * 512, 512)],
                            start=True, stop=True)
                        p = ppool.tile([128, 512], BF16, tag="p",
                                       name=f"p{j}{hf}{kt}{g}")
                        if b_exp_dve:
                            nc.vector.tensor_copy(p, st)
                        elif not b_no_exp:
                            nc.scalar.activation(p, st, EXP, scale=0.125)
                        # causal mask: diagonal key tiles only
                        if kt >= 2 * j and not b_no_mask:
                            moff = 256 * (1 + kt - 2 * j)
                            mask_b = bass.AP(
                                tensor=msk_sb.tensor,
                                offset=msk_sb.offset + moff,
                                ap=[msk_sb.ap[0], [0, 2], [1, 256]])
                            eng = nc.gpsimd if mask_gpsimd else nc.vector
                            eng.tensor_mul(p, p, mask_b)
                        return p

                    prev = None
                    seen = 0
                    for batch in _batches(NKT):
                        cur = []
                        for idx, kt in enumerate(batch):
                            for g in range(2):
                                cur.append((kt, g, score(kt, g), seen == 0))
                            seen += 1
                            if idx == 0:
                                # PE work to overlap this batch's exps: the
                                # previous batch's AVs, or at a (j,hf)
                                # boundary the deferred tail of the previous
                                # block
                                if prev is not None:
                                    for pkt, g, p, first in prev:
                                        av1(pkt, g, p, first, False)
                                    if filler is not None:
                                        filler()
                                else:
                                    flush_b(filler)
                        prev = cur

                    def tail(prev=prev, av1=av1, norm=norm):
                        lastg = {}
                        for i, (pkt, g, p, first) in enumerate(prev):
                            lastg[g] = i
                        for i, (pkt, g, p, first) in enumerate(prev):
                            av1(pkt, g, p, first, lastg[g] == i)
                        norm()
                    carry[0] = tail

            def stage_otn(th):
                # odd-rep otn rows (otn2, partitions 0-63) -> otn rows 64-127
                # for this token half
                for oc in range(4):
                    nc.sync.dma_start(
                        out=otn_sb[64:128, ds(oc * 2048 + th * 1024, 1024)],
                        in_=otn2_sb[0:64, ds(oc * 2048 + th * 1024, 1024)])

            def c_block(th, oc):
                # post-B: K=128 full mode, yt rotating through both freed
                # bank sets so PE never waits on evacuation
                for qh in range(2):
                    tag = "st" if (oc * 2 + qh) % 2 == 0 else "og"
                    yt = ps.tile([128, 512], F32, tag=tag,
                                 bufs=(st_bufs if tag == "st" else og_bufs),
                                 name=f"yt{th}_{oc}_{qh}")
                    for ic in range(4):
                        nc.tensor.matmul(
                            yt,
                            wo_sb[:, ic * 2048 + oc * 128:
                                  ic * 2048 + (oc + 1) * 128],
                            otn_sb[:, ds(ic * 2048 + th * 1024 + qh * 512,
                                         512)],
                            start=(ic == 0), stop=(ic == 3))
                    yt_sb = ytp.tile([128, 512], BF16, tag="ytsb",
                                     name=f"ytsb{th}_{oc}_{qh}")
                    nc.vector.tensor_copy(yt_sb, yt)
                    nc.sync.dma_start(
                        out=YT[oc * 128:(oc + 1) * 128,
                               ds(th * 1024 + qh * 512, 512)],
                        in_=yt_sb)

            def body(_rep):
                # ---------------- input DMA ----------------
                # host pre-arranges every input into its SBUF layout; x/wq/
                # wk/wv stream in 4 ck-groups so phase A can start on group 0
                # while later groups are in flight; wo (phase C only) last.
                nc.sync.dma_start(out=msk_sb, in_=MSK4)
                for cg in range(4):
                    nc.sync.dma_start(out=x_sb[:, ds(cg * 8192, 8192)],
                                      in_=xT[:, ds(cg * 8192, 8192)])
                    nc.sync.dma_start(out=wq_sb[:, ds(cg * 2048, 2048)],
                                      in_=WqT[:, ds(cg * 2048, 2048)])
                    nc.sync.dma_start(out=wk_sb[:, ds(cg * 512, 512)],
                                      in_=WkT[:, ds(cg * 512, 512)])
                    nc.sync.dma_start(out=wv_sb[:, ds(cg * 512, 512)],
                                      in_=WvT[:, ds(cg * 512, 512)])
                nc.sync.dma_start(out=wo_sb, in_=WoT)

                # ---------------- A then B then C (serial phases; B is
                # internally pipelined and switch-free) -------------------
                A, Bp, C = ("A" in phases), ("B" in phases), ("C" in phases)
                if Bp and not A:   # diagnostic builds: satisfy the tracker
                    for t in (qt_sb, kt_bd, vt_sb, va_sb):
                        nc.vector.memset(t, 0.01)
                    if b_no_norm:
                        nc.vector.memset(otn_sb, 0.01)
                        nc.vector.memset(otn2_sb, 0.01)
                if A and not Bp:
                    for tb in range(4):
                        phase_a_wide(tb)
                if Bp:
                    # stagger: A(tb+1) is emitted between B(2tb-2,2tb-1) and
                    # B(2tb,2tb+1) -- its PE matmuls hide the earlier pair's
                    # exp stream (same tiling mode, shared tag rotation)
                    if A:
                        phase_a_wide(0)
                    phase_b(0)
                    phase_b(1)
                    if A:
                        phase_a_wide(1)
                    phase_b(2)
                    phase_b(3)
                    flush_b()
                    stage_otn(0)
                    if A:
                        phase_a_wide(2)
                    phase_b(4)
                    phase_b(5)
                    if A:
                        phase_a_wide(3)
                    # C token-half 0 fills B(6,7)'s exp-bound batch slots
                    pend = [(0, oc) for oc in range(16)] if C else []

                    def filler():
                        if pend:
                            c_block(*pend.pop(0))

                    phase_b(6, filler=filler)
                    phase_b(7, filler=filler)
                    flush_b()
                    while pend:
                        c_block(*pend.pop(0))
                elif C:
                    stage_otn(0)
                    for oc in range(16):
                        c_block(0, oc)
                if C:
                    stage_otn(1)
                    for oc in range(16):
                        c_block(1, oc)

            loop(loop_n, body)

    nc.compile()
    return nc


def _get_nc():
    if "nc" not in _CACHE:
        _CACHE["nc"] = _build_nc()
    return _CACHE["nc"]


# --------------------------------------------------------------------------
# host wrapper
# --------------------------------------------------------------------------
def _bf16(a):
    import ml_dtypes
    return np.ascontiguousarray(np.asarray(a).astype(ml_dtypes.bfloat16))


def _make_mask() -> np.ndarray:
    """4 multiplicative 0/1 blocks of [128, 256] (broadcast over reps):
    block 0: all-pass; 1: diag kt==2j; 2: diag kt==2j+1; 3: all-blocked."""
    ki = np.arange(128)[:, None]
    qi = np.arange(256)[None, :]
    o = np.ones((128, 256), np.float32)
    m0 = np.where(ki <= qi, 1.0, 0.0).astype(np.float32)
    m1 = np.where(128 + ki <= qi, 1.0, 0.0).astype(np.float32)
    mf = np.zeros((128, 256), np.float32)
    return np.concatenate([o, m0, m1, mf], axis=1)  # [128, 1024]


def _core_inputs(x, Wq, Wk, Wv, Wo, c, mask):
    b, hb = c // 4, c % 4
    xT_c = np.ascontiguousarray(x[b].T)
    # interleave q heads: chunk qc = [g0 rep qc (64) | g1 rep qc (64)]
    g0, g1 = 2 * hb, 2 * hb + 1
    cols = []
    for qc in range(NREP):
        cols.append(Wq[g0 * 256 + qc * 64: g0 * 256 + (qc + 1) * 64])
        cols.append(Wq[g1 * 256 + qc * 64: g1 * 256 + (qc + 1) * 64])
    WqT_c = np.ascontiguousarray(np.concatenate(cols, axis=0).T)
    WkT_c = np.ascontiguousarray(Wk[128 * hb:128 * (hb + 1)].T)
    WvT_c = np.ascontiguousarray(Wv[128 * hb:128 * (hb + 1)].T)
    WoT_c = np.ascontiguousarray(Wo[:, 512 * hb:512 * (hb + 1)].T)
    def _sb(a, nchunk):    # [nchunk*128, w] -> [128, nchunk*w] (ck-major cols)
        n = a.shape[0] // 128
        assert n == nchunk
        return a.reshape(n, 128, a.shape[1]).transpose(1, 0, 2).reshape(
            128, n * a.shape[1])
    return {"xT": _bf16(_sb(xT_c, 16)), "WqT": _bf16(_sb(WqT_c, 16)),
            "WkT": _bf16(_sb(WkT_c, 16)), "WvT": _bf16(_sb(WvT_c, 16)),
            "WoT": _bf16(_sb(WoT_c, 4)), "MSK4": _bf16(mask)}


def kernel(x, Wq, Wk, Wv, Wo, _trace=False, _trace_kwargs=None):
    from concourse import bass_utils

    x = np.asarray(x, dtype=np.float32)
    Wq = np.asarray(Wq, dtype=np.float32)
    Wk = np.asarray(Wk, dtype=np.float32)
    Wv = np.asarray(Wv, dtype=np.float32)
    Wo = np.asarray(Wo, dtype=np.float32)

    nc = _get_nc()
    mask = _make_mask()
    in_maps = [_core_inputs(x, Wq, Wk, Wv, Wo, c, mask) for c in range(8)]

    res = None
    last_exc = None
    for _attempt in range(3):
        try:
            res = bass_utils.run_bass_kernel_spmd(
                nc, in_maps, core_ids=list(range(8)),
                trace=_trace, **(_trace_kwargs or {}))
            break
        except Exception as e:  # transient device wedges happen; retry
            last_exc = e
    if res is None:
        raise last_exc

    Y = np.zeros((B, T, D_MODEL), dtype=np.float32)
    for c in range(8):
        Y[c // 4] += res.results[c]["YT"].T.astype(np.float32)
    if _trace:
        _CACHE["last_result"] = res
    return Y


# revision 61
# speedup vs baseline: 1.0417x; 1.0417x over previous
"""Grouped-query attention (B=2, T=2048, d_model=2048, 32 Q heads / 8 KV heads)
sharded over 8 NeuronCores: batch x head-block tensor parallel.

Core c handles batch b = c//4 and head-block hb = c%4 (8 q heads = 2 kv groups).
bf16 matmul operands everywhere (fp32 PSUM accumulate); host feeds pre-transposed
bf16 inputs and sums/transposes per-core partials.

v5: fully software-pipelined A/B/C on 1-bank PSUM tiles.
  - All PSUM accumulators are [128, 512] (one bank), so projections (A),
    attention (B) and output projection (C) can be co-resident in the 8 banks
    and overlap: A(tb) interleaves with B(2tb-2, 2tb-1); C's first token half
    fills PE slack inside B(6,7)'s exp-bound stretch; C's second half is the
    tail. B is ACT(exp)-bound, A/C are pure PE, so the stagger hides most of
    the exp cost.
  - Phase B processes query blocks j of 256 rows x 2 rep-halves; per half it
    scans exactly the causal prefix of key tiles (NKT = 2j+2) in batches of
    up to 4 (one PE tiling-mode round trip per batch). Score matmuls for the
    two kv groups auto-row-tile ((0,0)/(64,0) 64x128 PE tiles, concurrent).
    V_aug carries a 64-wide ones block so AV matmuls have M=128 and og rows
    64-127 hold the softmax denominator replicated across 64 partitions
    (vectorized reciprocal, no gpsimd broadcast). Causal masking:
    multiplicative 0/1 bf16 mask on P after exp, diagonal key tiles only.
  - Inputs stream in 4 ck-groups (phase A consumes group 0 while the rest
    are in flight); Wo loads last (phase C only).
"""

import numpy as np

D_MODEL = 2048
T = 2048
B = 2
DK = 64
NREP = 4

_CACHE: dict = {}


# --------------------------------------------------------------------------
# device kernel
# --------------------------------------------------------------------------
def _build_nc(loop_n=1, unroll=False, phases="ABC", st_bufs=4, og_bufs=4,
              interleave=False, b_no_exp=False, b_no_mask=False,
              b_no_norm=False, diag_first=False, norm_copy_dve=True,
              mask_gpsimd=False, b_exp_dve=False, norm_mul_gpsimd=True):
    import concourse.bass as bass
    import concourse.mybir as mybir
    import concourse.tile as tile
    from concourse import bacc
    from concourse.masks import make_identity

    F32 = mybir.dt.float32
    BF16 = mybir.dt.bfloat16
    EXP = mybir.ActivationFunctionType.Exp
    ds = bass.ds

    nc = bacc.Bacc("TRN2", target_bir_lowering=False, debug=False)

    xT = nc.dram_tensor("xT", [128, 16 * 2048], BF16, kind="ExternalInput").ap()
    WqT = nc.dram_tensor("WqT", [128, 16 * 512], BF16, kind="ExternalInput").ap()
    WkT = nc.dram_tensor("WkT", [128, 16 * 128], BF16, kind="ExternalInput").ap()
    WvT = nc.dram_tensor("WvT", [128, 16 * 128], BF16, kind="ExternalInput").ap()
    WoT = nc.dram_tensor("WoT", [128, 4 * 2048], BF16, kind="ExternalInput").ap()
    MSK4 = nc.dram_tensor("MSK4", [128, 1024], BF16, kind="ExternalInput").ap()
    YT = nc.dram_tensor("YT", [2048, 2048], BF16, kind="ExternalOutput").ap()

    with tile.TileContext(nc) as tc:
        def loop(n, body, **kw):
            """Hardware For_i over range(n), or python-unrolled (for the
            timeline simulator, which can't resolve reg-mode branches)."""
            if unroll:
                for v in range(n):
                    body(v)
            else:
                with tc.For_i(0, n, 1, **kw) as v:
                    body(v)

        with tc.tile_pool(name="consts", bufs=1) as consts, \
             tc.tile_pool(name="wts", bufs=1) as wts, \
             tc.tile_pool(name="persist", bufs=1) as persist, \
             tc.tile_pool(name="pp", bufs=8) as ppool, \
             tc.tile_pool(name="rcp", bufs=4) as rcp, \
             tc.tile_pool(name="ytp", bufs=4) as ytp, \
             tc.tile_pool(name="mrg", bufs=6) as mrg, \
             tc.tile_pool(name="ps", bufs=1, space="PSUM") as ps:

            # ---------------- constants (outside the timing loop) ----------
            idl_f32 = consts.tile([128, 128], F32)
            make_identity(nc, idl_f32)
            idl = consts.tile([128, 128], BF16)
            nc.vector.tensor_copy(idl, idl_f32)

            x_sb = wts.tile([128, 16 * 2048], BF16)   # col = ck*2048 + tok
            wq_sb = wts.tile([128, 16 * 512], BF16)   # col = ck*512 + qout
            wk_sb = wts.tile([128, 16 * 128], BF16)   # col = ck*128 + kout
            wv_sb = wts.tile([128, 16 * 128], BF16)
            wo_sb = wts.tile([128, 4 * 2048], BF16)   # col = ic*2048 + out
            msk_sb = consts.tile([128, 1024], BF16)   # 4 x [128,256] mask blocks

            qt_sb = persist.tile([128, 8192], BF16)   # col = j*1024 + r*256 + qi
            kt_bd = persist.tile([128, 4096], BF16)   # block-diag K tiles:
            # tile (kt,g) at cols (2kt+g)*128, K_g dk on rows 64g..64g+64,
            # zeros elsewhere (so K=128 score matmuls select one group)
            vt_sb = persist.tile([128, 2048], BF16)   # [kvd, tok]
            va_sb = persist.tile([128, 4096], BF16)   # 32 x [128 tok, 64 v | 64 ones]
            otn_sb = persist.tile([128, 8192], BF16)  # col = oc*2048 + tok
            otn2_sb = persist.tile([64, 8192], BF16)  # odd-rep rows, staged up

            nc.vector.memset(kt_bd, 0.0)   # zero rows persist across reps

            # ones block of every V_aug tile (values never change)
            ones_ap = bass.AP(tensor=va_sb.tensor, offset=va_sb.offset + 64,
                              ap=[va_sb.ap[0], [128, 32], [1, 64]])
            nc.vector.memset(ones_ap, 1.0)

            def _a_evacuate(tb, grp, acc):
                if grp < 4:      # q chunk: qt_sb col = j*1024 + qc*256 + qi
                    dst = bass.AP(
                        tensor=qt_sb.tensor,
                        offset=qt_sb.offset + tb * 2048 + grp * 256,
                        ap=[qt_sb.ap[0], [1024, 2], [1, 256]])
                    nc.vector.tensor_copy(dst, acc)
                elif grp == 5:
                    nc.vector.tensor_copy(vt_sb[:, ds(tb * 512, 512)], acc)
                else:            # K -> block-diagonal tiles
                    for g in range(2):
                        base = kt_bd[64 * g:64 * (g + 1), :]
                        dst = bass.AP(
                            tensor=base.tensor,
                            offset=base.offset + (8 * tb + g) * 128,
                            ap=[base.ap[0], [256, 4], [1, 128]])
                        nc.vector.tensor_copy(
                            dst, acc[64 * g:64 * (g + 1), :])

            def _a_w(grp, ck):
                if grp < 4:
                    return wq_sb[:, ck * 512 + grp * 128:
                                 ck * 512 + (grp + 1) * 128]
                w_t = wk_sb if grp == 4 else wv_sb
                return w_t[:, ck * 128:(ck + 1) * 128]

            def _a_mm128(tb, grp, acc, ck):
                nc.tensor.matmul(
                    acc, _a_w(grp, ck),
                    x_sb[:, ds(tb * 512 + ck * 2048, 512)],
                    start=(ck == 0), stop=(ck == 15))

            def _a_vtp(tb):
                # V_aug build for this tb's 4 key tiles: transpose VT
                # 128-blocks into [tok, vdim] tiles
                for i in range(4):
                    kt = 4 * tb + i
                    vtp = ps.tile([128, 128], BF16, tag="pa", bufs=2,
                                  name=f"vtp{kt}")
                    nc.tensor.transpose(
                        vtp, vt_sb[:, kt * 128:(kt + 1) * 128], idl)
                    # vtp cols 0-63 = g0 vdims -> va tile kt; 64-127 = g1
                    # vdims -> va tile 16+kt (ones block at +64 untouched)
                    dest = bass.AP(tensor=va_sb.tensor,
                                   offset=va_sb.offset + kt * 128,
                                   ap=[va_sb.ap[0], [16 * 128, 2], [1, 64]])
                    src = bass.AP(tensor=vtp.tensor, offset=vtp.offset,
                                  ap=[vtp.ap[0], [64, 2], [1, 64]])
                    nc.vector.tensor_copy(dest, src)

            def phase_a_wide(tb):
                # pre-B: all 6 accumulation groups in flight, ck-major, so
                # the PE tracks the input DMA stream; K=128 full mode (the
                # one tiling-mode switch happens at the A->B boundary)
                tags = ["og", "og", "og", "og", "st", "st"]
                accs = [ps.tile([128, 512], F32, tag=tags[grp],
                                bufs=(og_bufs if grp < 4 else st_bufs),
                                name=f"aw{tb}_{grp}") for grp in range(6)]
                for ck in range(16):
                    for grp in range(6):
                        _a_mm128(tb, grp, accs[grp], ck)
                for grp in range(6):
                    _a_evacuate(tb, grp, accs[grp])
                for i in range(4):
                    kt = 4 * tb + i
                    vtp = ps.tile([128, 128], BF16, tag="og", bufs=og_bufs,
                                  name=f"vtp{kt}")
                    nc.tensor.transpose(
                        vtp, vt_sb[:, kt * 128:(kt + 1) * 128], idl)
                    dest = bass.AP(tensor=va_sb.tensor,
                                   offset=va_sb.offset + kt * 128,
                                   ap=[va_sb.ap[0], [16 * 128, 2], [1, 64]])
                    src2 = bass.AP(tensor=vtp.tensor, offset=vtp.offset,
                                   ap=[vtp.ap[0], [64, 2], [1, 64]])
                    nc.vector.tensor_copy(dest, src2)

            def phase_a_wide(tb):
                # pre-B: all 6 accumulation groups in flight, ck-major, so
                # the PE tracks the input DMA stream; K=128 full mode (the
                # one tiling-mode switch happens at the A->B boundary)
                tags = ["og", "og", "og", "og", "st", "st"]
                accs = [ps.tile([128, 512], F32, tag=tags[grp],
                                bufs=(og_bufs if grp < 4 else st_bufs),
                                name=f"aw{tb}_{grp}") for grp in range(6)]
                for ck in range(16):
                    for grp in range(6):
                        _a_mm128(tb, grp, accs[grp], ck)
                for grp in range(6):
                    _a_evacuate(tb, grp, accs[grp])
                for i in range(4):
                    kt = 4 * tb + i
                    vtp = ps.tile([128, 128], BF16, tag="og", bufs=og_bufs,
                                  name=f"vtp{kt}")
                    nc.tensor.transpose(
                        vtp, vt_sb[:, kt * 128:(kt + 1) * 128], idl)
                    dest = bass.AP(tensor=va_sb.tensor,
                                   offset=va_sb.offset + kt * 128,
                                   ap=[va_sb.ap[0], [16 * 128, 2], [1, 64]])
                    src2 = bass.AP(tensor=vtp.tensor, offset=vtp.offset,
                                   ap=[vtp.ap[0], [64, 2], [1, 64]])
                    nc.vector.tensor_copy(dest, src2)

            def a_chunks(tb):
                """Phase A(tb) as background chunks: 6 accumulation groups
                (16 matmul pairs + merge-evac each) plus the V_aug
                transposes. Emitted one per phase-B batch slot so A's PE
                work fills B's exp-bound slack."""
                def grp_chunk(grp):
                    def run():
                        acc2 = [ps.tile([128, 512], F32, tag="pa", bufs=2,
                                        name=f"pa{tb}_{grp}_{hh}")
                                for hh in range(2)]
                        for ck in range(16):
                            _a_mm64(tb, grp, acc2, ck)
                        tmp = mrg.tile([128, 512], F32, tag="mrg",
                                       name=f"mt{tb}_{grp}")
                        nc.scalar.copy(tmp, acc2[1])
                        nc.vector.tensor_add(
                            qt_sb[:, 0:1], acc2[0], tmp)  # unused path
                    return run
                return ([("A", tb, grp_chunk(g)) for g in range(6)]
                        + [("A", tb, lambda: _a_vtp(tb))])

            def phase_a0():
                # ck-major variant for tb=0: all 6 accumulation groups in
                # flight (B hasn't started; its st/og banks are free), so the
                # PE tracks the input DMA stream instead of stalling on two
                # serialized groups.
                tags = ["pa", "pa", "st", "st", "og", "og"]
                tbufs = {"pa": 2, "st": st_bufs, "og": og_bufs}
                accs = [ps.tile([128, 512], F32, tag=tags[grp],
                                bufs=tbufs[tags[grp]],
                                name=f"pa0_{grp}") for grp in range(6)]
                for ck in range(16):
                    for grp in range(6):
                        _a_mm128(0, grp, accs[grp], ck)
                for grp in range(6):
                    _a_evacuate(tb, grp, accs[grp])
                for i in range(4):
                    vtp = ps.tile([128, 128], BF16, tag="pa", bufs=2,
                                  name=f"vtp0_{i}")
                    nc.tensor.transpose(
                        vtp, vt_sb[:, i * 128:(i + 1) * 128], idl)
                    dest = bass.AP(tensor=va_sb.tensor,
                                   offset=va_sb.offset + i * 128,
                                   ap=[va_sb.ap[0], [16 * 128, 2], [1, 64]])
                    src = bass.AP(tensor=vtp.tensor, offset=vtp.offset,
                                  ap=[vtp.ap[0], [64, 2], [1, 64]])
                    nc.vector.tensor_copy(dest, src)

            def _batches(nkt):
                """Key-tile batches of up to 4, DIAGONAL TILES FIRST: their
                mask multiplies then overlap later tiles' scores/exps
                instead of serializing the block tail (AV accumulation
                order is arbitrary)."""
                order = ([nkt - 2, nkt - 1] + list(range(nkt - 2))
                         if diag_first else list(range(nkt)))
                return [order[i:i + 4] for i in range(0, nkt, 4)]

            carry = [None]   # deferred tail (last AVs + normalize) per (j,hf)

            def flush_b(filler=None):
                if carry[0] is not None:
                    carry[0]()
                    carry[0] = None
                    if filler is not None:
                        filler()

            def phase_b(j, filler=None):
                """Everything in here runs in the PE's 64x128 tiling mode:
                scores are (0,0)/(64,0) row-tile pairs, and AV matmuls split
                their 128-token contraction into two 64-row halves
                accumulating into separate og banks (merged by the DVE in
                the normalize step). No tiling-mode drains in the loop.

                Each (j,hf)'s last AV batch + normalize is deferred and
                emitted inside the NEXT (j,hf)'s first batch, right after
                its first score pair, so the PE never sits at a boundary."""
                NKT = 2 * j + 2
                for hf in range(2):     # rep-halves of the 1024-wide q block
                    og = [ps.tile([128, 512], F32, tag="og", bufs=og_bufs,
                                  name=f"og{j}_{hf}_{g}") for g in range(2)]

                    def av1(pkt, g, p, first, last, og=og):
                        nc.tensor.matmul(
                            og[g],
                            va_sb[:, (g * 16 + pkt) * 128:
                                  (g * 16 + pkt) * 128 + 128],
                            p, start=first, stop=last)

                    def norm(og=og, j=j, hf=hf):
                        if b_no_norm:
                            return
                        # og rows 64-127 hold the denominator replicated
                        # across 64 partitions; recip and the normalize
                        # muls read og directly (one PSUM input each). og
                        # is double-buffered across (j,hf) blocks, so this
                        # never stalls the next block's AV matmuls.
                        for g in range(2):
                            rec = rcp.tile([64, 512], F32, tag="rec",
                                           name=f"r{j}{hf}{g}")
                            nc.vector.reciprocal(rec, og[g][64:128, :])
                            oc = 2 * g + hf
                            for r in range(2):   # rep 2hf, 2hf+1
                                dst = otn_sb if r == 0 else otn2_sb
                                nc.vector.tensor_mul(
                                    dst[0:64, ds(oc * 2048 + j * 256, 256)],
                                    og[g][0:64, r * 256:(r + 1) * 256],
                                    rec[:, r * 256:(r + 1) * 256])

                    def score(kt, g):
                        # K=128 full-mode matmul; the block-diagonal zero
                        # rows of the weight tile select group g from the
                        # stacked Q (no tiling-mode changes anywhere)
                        st = ps.tile([128, 512], F32, tag="st",
                                     bufs=st_bufs, name=f"st{j}{hf}{kt}{g}")
                        nc.tensor.matmul(
                            st,
                            kt_bd[:, (2 * kt + g) * 128:
                                  (2 * kt + g + 1) * 128],
                            qt_sb[:, ds(j * 1024 + hf * 512, 512)],
                            start=True, stop=True)
                        p = ppool.tile([128, 512], BF16, tag="p",
                                       name=f"p{j}{hf}{kt}{g}")
                        if b_exp_dve:
                            nc.vector.tensor_copy(p, st)
                        elif not b_no_exp:
                            nc.scalar.activation(p, st, EXP, scale=0.125)
                        # causal mask: diagonal key tiles only
                        if kt >= 2 * j and not b_no_mask:
                            moff = 256 * (1 + kt - 2 * j)
                            mask_b = bass.AP(
                                tensor=msk_sb.tensor,
                                offset=msk_sb.offset + moff,
                                ap=[msk_sb.ap[0], [0, 2], [1, 256]])
                            eng = nc.gpsimd if mask_gpsimd else nc.vector
                            eng.tensor_mul(p, p, mask_b)
                        return p

                    prev = None
                    seen = 0
                    for batch in _batches(NKT):
                        cur = []
                        for idx, kt in enumerate(batch):
                            for g in range(2):
                                cur.append((kt, g, score(kt, g), seen == 0))
                            seen += 1
                            if idx == 0:
                                # PE work to overlap this batch's exps: the
                                # previous batch's AVs, or at a (j,hf)
                                # boundary the deferred tail of the previous
                                # block
                                if prev is not None:
                                    for pkt, g, p, first in prev:
                                        av1(pkt, g, p, first, False)
                                    if filler is not None:
                                        filler()
                                else:
                                    flush_b(filler)
                        prev = cur

                    def tail(prev=prev, av1=av1, norm=norm):
                        lastg = {}
                        for i, (pkt, g, p, first) in enumerate(prev):
                            lastg[g] = i
                        for i, (pkt, g, p, first) in enumerate(prev):
                            av1(pkt, g, p, first, lastg[g] == i)
                        norm()
                    carry[0] = tail

            def stage_otn(th):
                # odd-rep otn rows (otn2, partitions 0-63) -> otn rows 64-127
                # for this token half
                for oc in range(4):
                    nc.sync.dma_start(
                        out=otn_sb[64:128, ds(oc * 2048 + th * 1024, 1024)],
                        in_=otn2_sb[0:64, ds(oc * 2048 + th * 1024, 1024)])

            def c_block(th, oc):
                # post-B: K=128 full mode, yt rotating through both freed
                # bank sets so PE never waits on evacuation
                for qh in range(2):
                    tag = "st" if (oc * 2 + qh) % 2 == 0 else "og"
                    yt = ps.tile([128, 512], F32, tag=tag,
                                 bufs=(st_bufs if tag == "st" else og_bufs),
                                 name=f"yt{th}_{oc}_{qh}")
                    for ic in range(4):
                        nc.tensor.matmul(
                            yt,
                            wo_sb[:, ic * 2048 + oc * 128:
                                  ic * 2048 + (oc + 1) * 128],
                            otn_sb[:, ds(ic * 2048 + th * 1024 + qh * 512,
                                         512)],
                            start=(ic == 0), stop=(ic == 3))
                    yt_sb = ytp.tile([128, 512], BF16, tag="ytsb",
                                     name=f"ytsb{th}_{oc}_{qh}")
                    nc.vector.tensor_copy(yt_sb, yt)
                    nc.sync.dma_start(
                        out=YT[oc * 128:(oc + 1) * 128,
                               ds(th * 1024 + qh * 512, 512)],
                        in_=yt_sb)

            def body(_rep):
                # ---------------- input DMA ----------------
                # host pre-arranges every input into its SBUF layout; x/wq/
                # wk/wv stream in 4 ck-groups so phase A can start on group 0
                # while later groups are in flight; wo (phase C only) last.
                nc.sync.dma_start(out=msk_sb, in_=MSK4)
                for cg in range(4):
                    nc.sync.dma_start(out=x_sb[:, ds(cg * 8192, 8192)],
                                      in_=xT[:, ds(cg * 8192, 8192)])
                    nc.sync.dma_start(out=wq_sb[:, ds(cg * 2048, 2048)],
                                      in_=WqT[:, ds(cg * 2048, 2048)])
                    nc.sync.dma_start(out=wk_sb[:, ds(cg * 512, 512)],
                                      in_=WkT[:, ds(cg * 512, 512)])
                    nc.sync.dma_start(out=wv_sb[:, ds(cg * 512, 512)],
                                      in_=WvT[:, ds(cg * 512, 512)])
                nc.sync.dma_start(out=wo_sb, in_=WoT)

                # ---------------- A then B then C (serial phases; B is
                # internally pipelined and switch-free) -------------------
                A, Bp, C = ("A" in phases), ("B" in phases), ("C" in phases)
                if Bp and not A:   # diagnostic builds: satisfy the tracker
                    for t in (qt_sb, kt_bd, vt_sb, va_sb):
                        nc.vector.memset(t, 0.01)
                    if b_no_norm:
                        nc.vector.memset(otn_sb, 0.01)
                        nc.vector.memset(otn2_sb, 0.01)
                if A:
                    for tb in range(4):
                        phase_a_wide(tb)
                if Bp:
                    phase_b(0)
                    phase_b(1)
                    phase_b(2)
                    phase_b(3)
                    flush_b()
                    stage_otn(0)
                    phase_b(4)
                    phase_b(5)
                    # C token-half 0 fills B(6,7)'s exp-bound batch slots
                    pend = [(0, oc) for oc in range(16)] if C else []

                    def filler():
                        if pend:
                            c_block(*pend.pop(0))

                    phase_b(6, filler=filler)
                    phase_b(7, filler=filler)
                    flush_b()
                    while pend:
                        c_block(*pend.pop(0))
                elif C:
                    stage_otn(0)
                    for oc in range(16):
                        c_block(0, oc)
                if C:
                    stage_otn(1)
                    for oc in range(16):
                        c_block(1, oc)

            loop(loop_n, body)

    nc.compile()
    return nc


def _get_nc():
    if "nc" not in _CACHE:
        _CACHE["nc"] = _build_nc()
    return _CACHE["nc"]


# --------------------------------------------------------------------------
# host wrapper
# --------------------------------------------------------------------------
def _bf16(a):
    import ml_dtypes
    return np.ascontiguousarray(np.asarray(a).astype(ml_dtypes.bfloat16))


def _make_mask() -> np.ndarray:
    """4 multiplicative 0/1 blocks of [128, 256] (broadcast over reps):
    block 0: all-pass; 1: diag kt==2j; 2: diag kt==2j+1; 3: all-blocked."""
    ki = np.arange(128)[:, None]
    qi = np.arange(256)[None, :]
    o = np.ones((128, 256), np.float32)
    m0 = np.where(ki <= qi, 1.0, 0.0).astype(np.float32)
    m1 = np.where(128 + ki <= qi, 1.0, 0.0).astype(np.float32)
    mf = np.zeros((128, 256), np.float32)
    return np.concatenate([o, m0, m1, mf], axis=1)  # [128, 1024]


def _core_inputs(x, Wq, Wk, Wv, Wo, c, mask):
    b, hb = c // 4, c % 4
    xT_c = np.ascontiguousarray(x[b].T)
    # interleave q heads: chunk qc = [g0 rep qc (64) | g1 rep qc (64)]
    g0, g1 = 2 * hb, 2 * hb + 1
    cols = []
    for qc in range(NREP):
        cols.append(Wq[g0 * 256 + qc * 64: g0 * 256 + (qc + 1) * 64])
        cols.append(Wq[g1 * 256 + qc * 64: g1 * 256 + (qc + 1) * 64])
    WqT_c = np.ascontiguousarray(np.concatenate(cols, axis=0).T)
    WkT_c = np.ascontiguousarray(Wk[128 * hb:128 * (hb + 1)].T)
    WvT_c = np.ascontiguousarray(Wv[128 * hb:128 * (hb + 1)].T)
    WoT_c = np.ascontiguousarray(Wo[:, 512 * hb:512 * (hb + 1)].T)
    def _sb(a, nchunk):    # [nchunk*128, w] -> [128, nchunk*w] (ck-major cols)
        n = a.shape[0] // 128
        assert n == nchunk
        return a.reshape(n, 128, a.shape[1]).transpose(1, 0, 2).reshape(
            128, n * a.shape[1])
    return {"xT": _bf16(_sb(xT_c, 16)), "WqT": _bf16(_sb(WqT_c, 16)),
            "WkT": _bf16(_sb(WkT_c, 16)), "WvT": _bf16(_sb(WvT_c, 16)),
            "WoT": _bf16(_sb(WoT_c, 4)), "MSK4": _bf16(mask)}


def kernel(x, Wq, Wk, Wv, Wo, _trace=False, _trace_kwargs=None):
    from concourse import bass_utils

    x = np.asarray(x, dtype=np.float32)
    Wq = np.asarray(Wq, dtype=np.float32)
    Wk = np.asarray(Wk, dtype=np.float32)
    Wv = np.asarray(Wv, dtype=np.float32)
    Wo = np.asarray(Wo, dtype=np.float32)

    nc = _get_nc()
    mask = _make_mask()
    in_maps = [_core_inputs(x, Wq, Wk, Wv, Wo, c, mask) for c in range(8)]

    res = None
    last_exc = None
    for _attempt in range(3):
        try:
            res = bass_utils.run_bass_kernel_spmd(
                nc, in_maps, core_ids=list(range(8)),
                trace=_trace, **(_trace_kwargs or {}))
            break
        except Exception as e:  # transient device wedges happen; retry
            last_exc = e
    if res is None:
        raise last_exc

    Y = np.zeros((B, T, D_MODEL), dtype=np.float32)
    for c in range(8):
        Y[c // 4] += res.results[c]["YT"].T.astype(np.float32)
    if _trace:
        _CACHE["last_result"] = res
    return Y


# revision 62
# speedup vs baseline: 1.0990x; 1.0550x over previous
"""Grouped-query attention (B=2, T=2048, d_model=2048, 32 Q heads / 8 KV heads)
sharded over 8 NeuronCores: batch x head-block tensor parallel.

Core c handles batch b = c//4 and head-block hb = c%4 (8 q heads = 2 kv groups).
bf16 matmul operands everywhere (fp32 PSUM accumulate); host feeds pre-transposed
bf16 inputs and sums/transposes per-core partials.

v5: fully software-pipelined A/B/C on 1-bank PSUM tiles.
  - All PSUM accumulators are [128, 512] (one bank), so projections (A),
    attention (B) and output projection (C) can be co-resident in the 8 banks
    and overlap: A(tb) interleaves with B(2tb-2, 2tb-1); C's first token half
    fills PE slack inside B(6,7)'s exp-bound stretch; C's second half is the
    tail. B is ACT(exp)-bound, A/C are pure PE, so the stagger hides most of
    the exp cost.
  - Phase B processes query blocks j of 256 rows x 2 rep-halves; per half it
    scans exactly the causal prefix of key tiles (NKT = 2j+2) in batches of
    up to 4 (one PE tiling-mode round trip per batch). Score matmuls for the
    two kv groups auto-row-tile ((0,0)/(64,0) 64x128 PE tiles, concurrent).
    V_aug carries a 64-wide ones block so AV matmuls have M=128 and og rows
    64-127 hold the softmax denominator replicated across 64 partitions
    (vectorized reciprocal, no gpsimd broadcast). Causal masking:
    multiplicative 0/1 bf16 mask on P after exp, diagonal key tiles only.
  - Inputs stream in 4 ck-groups (phase A consumes group 0 while the rest
    are in flight); Wo loads last (phase C only).
"""

import numpy as np

D_MODEL = 2048
T = 2048
B = 2
DK = 64
NREP = 4

_CACHE: dict = {}


# --------------------------------------------------------------------------
# device kernel
# --------------------------------------------------------------------------
def _build_nc(loop_n=1, unroll=False, phases="ABC", st_bufs=4, og_bufs=4,
              interleave=False, b_no_exp=False, b_no_mask=False,
              b_no_norm=False, diag_first=False, norm_copy_dve=True,
              mask_gpsimd=False, b_exp_dve=False, norm_mul_gpsimd=True):
    import concourse.bass as bass
    import concourse.mybir as mybir
    import concourse.tile as tile
    from concourse import bacc
    from concourse.masks import make_identity

    F32 = mybir.dt.float32
    BF16 = mybir.dt.bfloat16
    EXP = mybir.ActivationFunctionType.Exp
    ds = bass.ds

    nc = bacc.Bacc("TRN2", target_bir_lowering=False, debug=False)

    xT = nc.dram_tensor("xT", [128, 16 * 2048], BF16, kind="ExternalInput").ap()
    WqT = nc.dram_tensor("WqT", [128, 16 * 512], BF16, kind="ExternalInput").ap()
    WkT = nc.dram_tensor("WkT", [128, 16 * 128], BF16, kind="ExternalInput").ap()
    WvT = nc.dram_tensor("WvT", [128, 16 * 128], BF16, kind="ExternalInput").ap()
    WoT = nc.dram_tensor("WoT", [128, 4 * 2048], BF16, kind="ExternalInput").ap()
    MSK4 = nc.dram_tensor("MSK4", [128, 1024], BF16, kind="ExternalInput").ap()
    YT = nc.dram_tensor("YT", [2048, 2048], BF16, kind="ExternalOutput").ap()

    with tile.TileContext(nc) as tc:
        def loop(n, body, **kw):
            """Hardware For_i over range(n), or python-unrolled (for the
            timeline simulator, which can't resolve reg-mode branches)."""
            if unroll:
                for v in range(n):
                    body(v)
            else:
                with tc.For_i(0, n, 1, **kw) as v:
                    body(v)

        with tc.tile_pool(name="consts", bufs=1) as consts, \
             tc.tile_pool(name="wts", bufs=1) as wts, \
             tc.tile_pool(name="persist", bufs=1) as persist, \
             tc.tile_pool(name="pp", bufs=8) as ppool, \
             tc.tile_pool(name="rcp", bufs=4) as rcp, \
             tc.tile_pool(name="ytp", bufs=4) as ytp, \
             tc.tile_pool(name="mrg", bufs=6) as mrg, \
             tc.tile_pool(name="ps", bufs=1, space="PSUM") as ps:

            # ---------------- constants (outside the timing loop) ----------
            idl_f32 = consts.tile([128, 128], F32)
            make_identity(nc, idl_f32)
            idl = consts.tile([128, 128], BF16)
            nc.vector.tensor_copy(idl, idl_f32)

            x_sb = wts.tile([128, 16 * 2048], BF16)   # col = ck*2048 + tok
            wq_sb = wts.tile([128, 16 * 512], BF16)   # col = ck*512 + qout
            wk_sb = wts.tile([128, 16 * 128], BF16)   # col = ck*128 + kout
            wv_sb = wts.tile([128, 16 * 128], BF16)
            wo_sb = wts.tile([128, 4 * 2048], BF16)   # col = ic*2048 + out
            msk_sb = consts.tile([128, 1024], BF16)   # 4 x [128,256] mask blocks

            qt_sb = persist.tile([128, 8192], BF16)   # col = j*1024 + r*256 + qi
            kt_bd = persist.tile([128, 4096], BF16)   # block-diag K tiles:
            # tile (kt,g) at cols (2kt+g)*128, K_g dk on rows 64g..64g+64,
            # zeros elsewhere (so K=128 score matmuls select one group)
            vt_sb = persist.tile([128, 2048], BF16)   # [kvd, tok]
            va_sb = persist.tile([128, 4096], BF16)   # 32 x [128 tok, 64 v | 64 ones]
            otn_sb = persist.tile([128, 8192], BF16)  # col = oc*2048 + tok
            otn2_sb = persist.tile([64, 8192], BF16)  # odd-rep rows, staged up

            nc.vector.memset(kt_bd, 0.0)   # zero rows persist across reps

            # ones block of every V_aug tile (values never change)
            ones_ap = bass.AP(tensor=va_sb.tensor, offset=va_sb.offset + 64,
                              ap=[va_sb.ap[0], [128, 32], [1, 64]])
            nc.vector.memset(ones_ap, 1.0)

            def _a_evacuate(tb, grp, acc):
                if grp < 4:      # q chunk: qt_sb col = j*1024 + qc*256 + qi
                    dst = bass.AP(
                        tensor=qt_sb.tensor,
                        offset=qt_sb.offset + tb * 2048 + grp * 256,
                        ap=[qt_sb.ap[0], [1024, 2], [1, 256]])
                    nc.vector.tensor_copy(dst, acc)
                elif grp == 5:
                    nc.vector.tensor_copy(vt_sb[:, ds(tb * 512, 512)], acc)
                else:            # K -> block-diagonal tiles
                    for g in range(2):
                        base = kt_bd[64 * g:64 * (g + 1), :]
                        dst = bass.AP(
                            tensor=base.tensor,
                            offset=base.offset + (8 * tb + g) * 128,
                            ap=[base.ap[0], [256, 4], [1, 128]])
                        nc.vector.tensor_copy(
                            dst, acc[64 * g:64 * (g + 1), :])

            def _a_w(grp, ck):
                if grp < 4:
                    return wq_sb[:, ck * 512 + grp * 128:
                                 ck * 512 + (grp + 1) * 128]
                w_t = wk_sb if grp == 4 else wv_sb
                return w_t[:, ck * 128:(ck + 1) * 128]

            def _a_mm128(tb, grp, acc, ck):
                nc.tensor.matmul(
                    acc, _a_w(grp, ck),
                    x_sb[:, ds(tb * 512 + ck * 2048, 512)],
                    start=(ck == 0), stop=(ck == 15))

            def _a_vtp(tb):
                # V_aug build for this tb's 4 key tiles: transpose VT
                # 128-blocks into [tok, vdim] tiles
                for i in range(4):
                    kt = 4 * tb + i
                    vtp = ps.tile([128, 128], BF16, tag="pa", bufs=2,
                                  name=f"vtp{kt}")
                    nc.tensor.transpose(
                        vtp, vt_sb[:, kt * 128:(kt + 1) * 128], idl)
                    # vtp cols 0-63 = g0 vdims -> va tile kt; 64-127 = g1
                    # vdims -> va tile 16+kt (ones block at +64 untouched)
                    dest = bass.AP(tensor=va_sb.tensor,
                                   offset=va_sb.offset + kt * 128,
                                   ap=[va_sb.ap[0], [16 * 128, 2], [1, 64]])
                    src = bass.AP(tensor=vtp.tensor, offset=vtp.offset,
                                  ap=[vtp.ap[0], [64, 2], [1, 64]])
                    nc.vector.tensor_copy(dest, src)

            def phase_a_wide(tb):
                # pre-B: all 6 accumulation groups in flight, ck-major, so
                # the PE tracks the input DMA stream; K=128 full mode (the
                # one tiling-mode switch happens at the A->B boundary)
                tags = ["og", "og", "og", "og", "st", "st"]
                accs = [ps.tile([128, 512], F32, tag=tags[grp],
                                bufs=(og_bufs if grp < 4 else st_bufs),
                                name=f"aw{tb}_{grp}") for grp in range(6)]
                for ck in range(16):
                    for grp in range(6):
                        _a_mm128(tb, grp, accs[grp], ck)
                for grp in range(6):
                    _a_evacuate(tb, grp, accs[grp])
                for i in range(4):
                    kt = 4 * tb + i
                    vtp = ps.tile([128, 128], BF16, tag="og", bufs=og_bufs,
                                  name=f"vtp{kt}")
                    nc.tensor.transpose(
                        vtp, vt_sb[:, kt * 128:(kt + 1) * 128], idl)
                    dest = bass.AP(tensor=va_sb.tensor,
                                   offset=va_sb.offset + kt * 128,
                                   ap=[va_sb.ap[0], [16 * 128, 2], [1, 64]])
                    src2 = bass.AP(tensor=vtp.tensor, offset=vtp.offset,
                                   ap=[vtp.ap[0], [64, 2], [1, 64]])
                    nc.vector.tensor_copy(dest, src2)

            def phase_a_wide(tb):
                # pre-B: all 6 accumulation groups in flight, ck-major, so
                # the PE tracks the input DMA stream; K=128 full mode (the
                # one tiling-mode switch happens at the A->B boundary)
                tags = ["og", "og", "og", "og", "st", "st"]
                accs = [ps.tile([128, 512], F32, tag=tags[grp],
                                bufs=(og_bufs if grp < 4 else st_bufs),
                                name=f"aw{tb}_{grp}") for grp in range(6)]
                for ck in range(16):
                    for grp in range(6):
                        _a_mm128(tb, grp, accs[grp], ck)
                for grp in range(6):
                    _a_evacuate(tb, grp, accs[grp])
                for i in range(4):
                    kt = 4 * tb + i
                    vtp = ps.tile([128, 128], BF16, tag="og", bufs=og_bufs,
                                  name=f"vtp{kt}")
                    nc.tensor.transpose(
                        vtp, vt_sb[:, kt * 128:(kt + 1) * 128], idl)
                    dest = bass.AP(tensor=va_sb.tensor,
                                   offset=va_sb.offset + kt * 128,
                                   ap=[va_sb.ap[0], [16 * 128, 2], [1, 64]])
                    src2 = bass.AP(tensor=vtp.tensor, offset=vtp.offset,
                                   ap=[vtp.ap[0], [64, 2], [1, 64]])
                    nc.vector.tensor_copy(dest, src2)

            def a_chunks(tb):
                """Phase A(tb) as background chunks: 6 accumulation groups
                (16 matmul pairs + merge-evac each) plus the V_aug
                transposes. Emitted one per phase-B batch slot so A's PE
                work fills B's exp-bound slack."""
                def grp_chunk(grp):
                    def run():
                        acc2 = [ps.tile([128, 512], F32, tag="pa", bufs=2,
                                        name=f"pa{tb}_{grp}_{hh}")
                                for hh in range(2)]
                        for ck in range(16):
                            _a_mm64(tb, grp, acc2, ck)
                        tmp = mrg.tile([128, 512], F32, tag="mrg",
                                       name=f"mt{tb}_{grp}")
                        nc.scalar.copy(tmp, acc2[1])
                        nc.vector.tensor_add(
                            qt_sb[:, 0:1], acc2[0], tmp)  # unused path
                    return run
                return ([("A", tb, grp_chunk(g)) for g in range(6)]
                        + [("A", tb, lambda: _a_vtp(tb))])

            def phase_a0():
                # ck-major variant for tb=0: all 6 accumulation groups in
                # flight (B hasn't started; its st/og banks are free), so the
                # PE tracks the input DMA stream instead of stalling on two
                # serialized groups.
                tags = ["pa", "pa", "st", "st", "og", "og"]
                tbufs = {"pa": 2, "st": st_bufs, "og": og_bufs}
                accs = [ps.tile([128, 512], F32, tag=tags[grp],
                                bufs=tbufs[tags[grp]],
                                name=f"pa0_{grp}") for grp in range(6)]
                for ck in range(16):
                    for grp in range(6):
                        _a_mm128(0, grp, accs[grp], ck)
                for grp in range(6):
                    _a_evacuate(tb, grp, accs[grp])
                for i in range(4):
                    vtp = ps.tile([128, 128], BF16, tag="pa", bufs=2,
                                  name=f"vtp0_{i}")
                    nc.tensor.transpose(
                        vtp, vt_sb[:, i * 128:(i + 1) * 128], idl)
                    dest = bass.AP(tensor=va_sb.tensor,
                                   offset=va_sb.offset + i * 128,
                                   ap=[va_sb.ap[0], [16 * 128, 2], [1, 64]])
                    src = bass.AP(tensor=vtp.tensor, offset=vtp.offset,
                                  ap=[vtp.ap[0], [64, 2], [1, 64]])
                    nc.vector.tensor_copy(dest, src)

            def _batches(nkt):
                """Key-tile batches of up to 4, DIAGONAL TILES FIRST: their
                mask multiplies then overlap later tiles' scores/exps
                instead of serializing the block tail (AV accumulation
                order is arbitrary)."""
                order = ([nkt - 2, nkt - 1] + list(range(nkt - 2))
                         if diag_first else list(range(nkt)))
                return [order[i:i + 4] for i in range(0, nkt, 4)]

            carry = [None]   # deferred tail (last AVs + normalize) per (j,hf)

            def flush_b(filler=None):
                if carry[0] is not None:
                    carry[0]()
                    carry[0] = None
                    if filler is not None:
                        filler()

            def phase_b(j, filler=None):
                """Everything in here runs in the PE's 64x128 tiling mode:
                scores are (0,0)/(64,0) row-tile pairs, and AV matmuls split
                their 128-token contraction into two 64-row halves
                accumulating into separate og banks (merged by the DVE in
                the normalize step). No tiling-mode drains in the loop.

                Each (j,hf)'s last AV batch + normalize is deferred and
                emitted inside the NEXT (j,hf)'s first batch, right after
                its first score pair, so the PE never sits at a boundary."""
                NKT = 2 * j + 2
                for hf in range(2):     # rep-halves of the 1024-wide q block
                    og = [ps.tile([128, 512], F32, tag="og", bufs=og_bufs,
                                  name=f"og{j}_{hf}_{g}") for g in range(2)]

                    def av1(pkt, g, p, first, last, og=og):
                        nc.tensor.matmul(
                            og[g],
                            va_sb[:, (g * 16 + pkt) * 128:
                                  (g * 16 + pkt) * 128 + 128],
                            p, start=first, stop=last)

                    def norm(og=og, j=j, hf=hf):
                        if b_no_norm:
                            return
                        # og rows 64-127 hold the denominator replicated
                        # across 64 partitions; recip and the normalize
                        # muls read og directly (one PSUM input each). og
                        # is double-buffered across (j,hf) blocks, so this
                        # never stalls the next block's AV matmuls.
                        for g in range(2):
                            rec = rcp.tile([64, 512], F32, tag="rec",
                                           name=f"r{j}{hf}{g}")
                            nc.vector.reciprocal(rec, og[g][64:128, :])
                            oc = 2 * g + hf
                            for r in range(2):   # rep 2hf, 2hf+1
                                dst = otn_sb if r == 0 else otn2_sb
                                nc.vector.tensor_mul(
                                    dst[0:64, ds(oc * 2048 + j * 256, 256)],
                                    og[g][0:64, r * 256:(r + 1) * 256],
                                    rec[:, r * 256:(r + 1) * 256])

                    def score(kt, g):
                        # K=128 full-mode matmul; the block-diagonal zero
                        # rows of the weight tile select group g from the
                        # stacked Q (no tiling-mode changes anywhere)
                        st = ps.tile([128, 512], F32, tag="st",
                                     bufs=st_bufs, name=f"st{j}{hf}{kt}{g}")
                        nc.tensor.matmul(
                            st,
                            kt_bd[:, (2 * kt + g) * 128:
                                  (2 * kt + g + 1) * 128],
                            qt_sb[:, ds(j * 1024 + hf * 512, 512)],
                            start=True, stop=True)
                        p = ppool.tile([128, 512], BF16, tag="p",
                                       name=f"p{j}{hf}{kt}{g}")
                        if b_exp_dve:
                            nc.vector.tensor_copy(p, st)
                        elif not b_no_exp:
                            nc.scalar.activation(p, st, EXP, scale=0.125)
                        # causal mask: diagonal key tiles only
                        if kt >= 2 * j and not b_no_mask:
                            moff = 256 * (1 + kt - 2 * j)
                            mask_b = bass.AP(
                                tensor=msk_sb.tensor,
                                offset=msk_sb.offset + moff,
                                ap=[msk_sb.ap[0], [0, 2], [1, 256]])
                            eng = nc.gpsimd if mask_gpsimd else nc.vector
                            eng.tensor_mul(p, p, mask_b)
                        return p

                    prev = None
                    seen = 0
                    for batch in _batches(NKT):
                        cur = []
                        for idx, kt in enumerate(batch):
                            for g in range(2):
                                cur.append((kt, g, score(kt, g), seen == 0))
                            seen += 1
                            if idx == 0:
                                # PE work to overlap this batch's exps: the
                                # previous batch's AVs, or at a (j,hf)
                                # boundary the deferred tail of the previous
                                # block
                                if prev is not None:
                                    for pkt, g, p, first in prev:
                                        av1(pkt, g, p, first, False)
                                    if filler is not None:
                                        filler()
                                else:
                                    flush_b(filler)
                        prev = cur

                    def tail(prev=prev, av1=av1, norm=norm):
                        lastg = {}
                        for i, (pkt, g, p, first) in enumerate(prev):
                            lastg[g] = i
                        for i, (pkt, g, p, first) in enumerate(prev):
                            av1(pkt, g, p, first, lastg[g] == i)
                        norm()
                    carry[0] = tail

            def stage_otn(th):
                # odd-rep otn rows (otn2, partitions 0-63) -> otn rows 64-127
                # for this token half
                for oc in range(4):
                    nc.sync.dma_start(
                        out=otn_sb[64:128, ds(oc * 2048 + th * 1024, 1024)],
                        in_=otn2_sb[0:64, ds(oc * 2048 + th * 1024, 1024)])

            def c_block(th, oc):
                # post-B: K=128 full mode, yt rotating through both freed
                # bank sets so PE never waits on evacuation
                for qh in range(2):
                    tag = "st" if (oc * 2 + qh) % 2 == 0 else "og"
                    yt = ps.tile([128, 512], F32, tag=tag,
                                 bufs=(st_bufs if tag == "st" else og_bufs),
                                 name=f"yt{th}_{oc}_{qh}")
                    for ic in range(4):
                        nc.tensor.matmul(
                            yt,
                            wo_sb[:, ic * 2048 + oc * 128:
                                  ic * 2048 + (oc + 1) * 128],
                            otn_sb[:, ds(ic * 2048 + th * 1024 + qh * 512,
                                         512)],
                            start=(ic == 0), stop=(ic == 3))
                    yt_sb = ytp.tile([128, 512], BF16, tag="ytsb",
                                     name=f"ytsb{th}_{oc}_{qh}")
                    nc.vector.tensor_copy(yt_sb, yt)
                    nc.sync.dma_start(
                        out=YT[oc * 128:(oc + 1) * 128,
                               ds(th * 1024 + qh * 512, 512)],
                        in_=yt_sb)

            def body(_rep):
                # ---------------- input DMA ----------------
                # host pre-arranges every input into its SBUF layout; x/wq/
                # wk/wv stream in 4 ck-groups so phase A can start on group 0
                # while later groups are in flight; wo (phase C only) last.
                nc.sync.dma_start(out=msk_sb, in_=MSK4)
                for cg in range(4):
                    nc.sync.dma_start(out=x_sb[:, ds(cg * 8192, 8192)],
                                      in_=xT[:, ds(cg * 8192, 8192)])
                    nc.sync.dma_start(out=wq_sb[:, ds(cg * 2048, 2048)],
                                      in_=WqT[:, ds(cg * 2048, 2048)])
                    nc.sync.dma_start(out=wk_sb[:, ds(cg * 512, 512)],
                                      in_=WkT[:, ds(cg * 512, 512)])
                    nc.sync.dma_start(out=wv_sb[:, ds(cg * 512, 512)],
                                      in_=WvT[:, ds(cg * 512, 512)])
                nc.sync.dma_start(out=wo_sb, in_=WoT)

                # ---------------- A then B then C (serial phases; B is
                # internally pipelined and switch-free) -------------------
                A, Bp, C = ("A" in phases), ("B" in phases), ("C" in phases)
                if Bp and not A:   # diagnostic builds: satisfy the tracker
                    for t in (qt_sb, kt_sb, vt_sb, va_sb):
                        nc.vector.memset(t, 0.01)
                    if b_no_norm:
                        nc.vector.memset(otn_sb, 0.01)
                        nc.vector.memset(otn2_sb, 0.01)
                if A:
                    for tb in range(4):
                        phase_a_wide(tb)
                if Bp:
                    for j in range(8):
                        phase_b(j)
                    flush_b()
                    stage_otn(0)
                elif C:
                    stage_otn(0)
                if C:
                    stage_otn(1)
                    for oc in range(16):
                        c_block(0, oc)
                    for oc in range(16):
                        c_block(1, oc)

            loop(loop_n, body)

    nc.compile()
    return nc


def _get_nc():
    if "nc" not in _CACHE:
        _CACHE["nc"] = _build_nc()
    return _CACHE["nc"]


# --------------------------------------------------------------------------
# host wrapper
# --------------------------------------------------------------------------
def _bf16(a):
    import ml_dtypes
    return np.ascontiguousarray(np.asarray(a).astype(ml_dtypes.bfloat16))


def _make_mask() -> np.ndarray:
    """4 multiplicative 0/1 blocks of [128, 256] (broadcast over reps):
    block 0: all-pass; 1: diag kt==2j; 2: diag kt==2j+1; 3: all-blocked."""
    ki = np.arange(128)[:, None]
    qi = np.arange(256)[None, :]
    o = np.ones((128, 256), np.float32)
    m0 = np.where(ki <= qi, 1.0, 0.0).astype(np.float32)
    m1 = np.where(128 + ki <= qi, 1.0, 0.0).astype(np.float32)
    mf = np.zeros((128, 256), np.float32)
    return np.concatenate([o, m0, m1, mf], axis=1)  # [128, 1024]


def _core_inputs(x, Wq, Wk, Wv, Wo, c, mask):
    b, hb = c // 4, c % 4
    xT_c = np.ascontiguousarray(x[b].T)
    # interleave q heads: chunk qc = [g0 rep qc (64) | g1 rep qc (64)]
    g0, g1 = 2 * hb, 2 * hb + 1
    cols = []
    for qc in range(NREP):
        cols.append(Wq[g0 * 256 + qc * 64: g0 * 256 + (qc + 1) * 64])
        cols.append(Wq[g1 * 256 + qc * 64: g1 * 256 + (qc + 1) * 64])
    WqT_c = np.ascontiguousarray(np.concatenate(cols, axis=0).T)
    WkT_c = np.ascontiguousarray(Wk[128 * hb:128 * (hb + 1)].T)
    WvT_c = np.ascontiguousarray(Wv[128 * hb:128 * (hb + 1)].T)
    WoT_c = np.ascontiguousarray(Wo[:, 512 * hb:512 * (hb + 1)].T)
    def _sb(a, nchunk):    # [nchunk*128, w] -> [128, nchunk*w] (ck-major cols)
        n = a.shape[0] // 128
        assert n == nchunk
        return a.reshape(n, 128, a.shape[1]).transpose(1, 0, 2).reshape(
            128, n * a.shape[1])
    return {"xT": _bf16(_sb(xT_c, 16)), "WqT": _bf16(_sb(WqT_c, 16)),
            "WkT": _bf16(_sb(WkT_c, 16)), "WvT": _bf16(_sb(WvT_c, 16)),
            "WoT": _bf16(_sb(WoT_c, 4)), "MSK4": _bf16(mask)}


def kernel(x, Wq, Wk, Wv, Wo, _trace=False, _trace_kwargs=None):
    from concourse import bass_utils

    x = np.asarray(x, dtype=np.float32)
    Wq = np.asarray(Wq, dtype=np.float32)
    Wk = np.asarray(Wk, dtype=np.float32)
    Wv = np.asarray(Wv, dtype=np.float32)
    Wo = np.asarray(Wo, dtype=np.float32)

    nc = _get_nc()
    mask = _make_mask()
    in_maps = [_core_inputs(x, Wq, Wk, Wv, Wo, c, mask) for c in range(8)]

    res = None
    last_exc = None
    for _attempt in range(3):
        try:
            res = bass_utils.run_bass_kernel_spmd(
                nc, in_maps, core_ids=list(range(8)),
                trace=_trace, **(_trace_kwargs or {}))
            break
        except Exception as e:  # transient device wedges happen; retry
            last_exc = e
    if res is None:
        raise last_exc

    Y = np.zeros((B, T, D_MODEL), dtype=np.float32)
    for c in range(8):
        Y[c // 4] += res.results[c]["YT"].T.astype(np.float32)
    if _trace:
        _CACHE["last_result"] = res
    return Y


# revision 63
# speedup vs baseline: 1.1513x; 1.0476x over previous
"""Grouped-query attention (B=2, T=2048, d_model=2048, 32 Q heads / 8 KV heads)
sharded over 8 NeuronCores: batch x head-block tensor parallel.

Core c handles batch b = c//4 and head-block hb = c%4 (8 q heads = 2 kv groups).
bf16 matmul operands everywhere (fp32 PSUM accumulate); host feeds pre-transposed
bf16 inputs and sums/transposes per-core partials.

v5: fully software-pipelined A/B/C on 1-bank PSUM tiles.
  - All PSUM accumulators are [128, 512] (one bank), so projections (A),
    attention (B) and output projection (C) can be co-resident in the 8 banks
    and overlap: A(tb) interleaves with B(2tb-2, 2tb-1); C's first token half
    fills PE slack inside B(6,7)'s exp-bound stretch; C's second half is the
    tail. B is ACT(exp)-bound, A/C are pure PE, so the stagger hides most of
    the exp cost.
  - Phase B processes query blocks j of 256 rows x 2 rep-halves; per half it
    scans exactly the causal prefix of key tiles (NKT = 2j+2) in batches of
    up to 4 (one PE tiling-mode round trip per batch). Score matmuls for the
    two kv groups auto-row-tile ((0,0)/(64,0) 64x128 PE tiles, concurrent).
    V_aug carries a 64-wide ones block so AV matmuls have M=128 and og rows
    64-127 hold the softmax denominator replicated across 64 partitions
    (vectorized reciprocal, no gpsimd broadcast). Causal masking:
    multiplicative 0/1 bf16 mask on P after exp, diagonal key tiles only.
  - Inputs stream in 4 ck-groups (phase A consumes group 0 while the rest
    are in flight); Wo loads last (phase C only).
"""

import numpy as np

D_MODEL = 2048
T = 2048
B = 2
DK = 64
NREP = 4

_CACHE: dict = {}


# --------------------------------------------------------------------------
# device kernel
# --------------------------------------------------------------------------
def _build_nc(loop_n=1, unroll=False, phases="ABC", st_bufs=4, og_bufs=4,
              interleave=False, b_no_exp=False, b_no_mask=False,
              b_no_norm=False, diag_first=False, norm_copy_dve=True,
              mask_gpsimd=False, b_exp_dve=False, norm_mul_gpsimd=True):
    import concourse.bass as bass
    import concourse.mybir as mybir
    import concourse.tile as tile
    from concourse import bacc
    from concourse.masks import make_identity

    F32 = mybir.dt.float32
    BF16 = mybir.dt.bfloat16
    EXP = mybir.ActivationFunctionType.Exp
    ds = bass.ds

    nc = bacc.Bacc("TRN2", target_bir_lowering=False, debug=False)

    xT = nc.dram_tensor("xT", [128, 16 * 2048], BF16, kind="ExternalInput").ap()
    WqT = nc.dram_tensor("WqT", [128, 16 * 512], BF16, kind="ExternalInput").ap()
    WkT = nc.dram_tensor("WkT", [128, 16 * 128], BF16, kind="ExternalInput").ap()
    WvT = nc.dram_tensor("WvT", [128, 16 * 128], BF16, kind="ExternalInput").ap()
    WoT = nc.dram_tensor("WoT", [128, 4 * 2048], BF16, kind="ExternalInput").ap()
    MSK4 = nc.dram_tensor("MSK4", [128, 1024], BF16, kind="ExternalInput").ap()
    YT = nc.dram_tensor("YT", [2048, 2048], BF16, kind="ExternalOutput").ap()

    with tile.TileContext(nc) as tc:
        def loop(n, body, **kw):
            """Hardware For_i over range(n), or python-unrolled (for the
            timeline simulator, which can't resolve reg-mode branches)."""
            if unroll:
                for v in range(n):
                    body(v)
            else:
                with tc.For_i(0, n, 1, **kw) as v:
                    body(v)

        with tc.tile_pool(name="consts", bufs=1) as consts, \
             tc.tile_pool(name="wts", bufs=1) as wts, \
             tc.tile_pool(name="persist", bufs=1) as persist, \
             tc.tile_pool(name="pp", bufs=8) as ppool, \
             tc.tile_pool(name="rcp", bufs=4) as rcp, \
             tc.tile_pool(name="ytp", bufs=4) as ytp, \
             tc.tile_pool(name="mrg", bufs=6) as mrg, \
             tc.tile_pool(name="ps", bufs=1, space="PSUM") as ps:

            # ---------------- constants (outside the timing loop) ----------
            idl_f32 = consts.tile([128, 128], F32)
            make_identity(nc, idl_f32)
            idl = consts.tile([128, 128], BF16)
            nc.vector.tensor_copy(idl, idl_f32)

            x_sb = wts.tile([128, 16 * 2048], BF16)   # col = ck*2048 + tok
            wq_sb = wts.tile([128, 16 * 512], BF16)   # col = ck*512 + qout
            wk_sb = wts.tile([128, 16 * 128], BF16)   # col = ck*128 + kout
            wv_sb = wts.tile([128, 16 * 128], BF16)
            wo_sb = wts.tile([128, 4 * 2048], BF16)   # col = ic*2048 + out
            msk_sb = consts.tile([128, 1024], BF16)   # 4 x [128,256] mask blocks

            qt_sb = persist.tile([128, 8192], BF16)   # col = j*1024 + r*256 + qi
            kt_bd = persist.tile([128, 4096], BF16)   # block-diag K tiles:
            # tile (kt,g) at cols (2kt+g)*128, K_g dk on rows 64g..64g+64,
            # zeros elsewhere (so K=128 score matmuls select one group)
            vt_sb = persist.tile([128, 2048], BF16)   # [kvd, tok]
            va_sb = persist.tile([128, 4096], BF16)   # 32 x [128 tok, 64 v | 64 ones]
            otn_sb = persist.tile([128, 8192], BF16)  # col = oc*2048 + tok
            otn2_sb = persist.tile([64, 8192], BF16)  # odd-rep rows, staged up

            nc.vector.memset(kt_bd, 0.0)   # zero rows persist across reps

            # ones block of every V_aug tile (values never change)
            ones_ap = bass.AP(tensor=va_sb.tensor, offset=va_sb.offset + 64,
                              ap=[va_sb.ap[0], [128, 32], [1, 64]])
            nc.vector.memset(ones_ap, 1.0)

            def _a_evacuate(tb, grp, acc):
                if grp < 4:      # q chunk: qt_sb col = j*1024 + qc*256 + qi
                    dst = bass.AP(
                        tensor=qt_sb.tensor,
                        offset=qt_sb.offset + tb * 2048 + grp * 256,
                        ap=[qt_sb.ap[0], [1024, 2], [1, 256]])
                    nc.vector.tensor_copy(dst, acc)
                elif grp == 5:
                    nc.vector.tensor_copy(vt_sb[:, ds(tb * 512, 512)], acc)
                else:            # K -> block-diagonal tiles
                    for g in range(2):
                        base = kt_bd[64 * g:64 * (g + 1), :]
                        dst = bass.AP(
                            tensor=base.tensor,
                            offset=base.offset + (8 * tb + g) * 128,
                            ap=[base.ap[0], [256, 4], [1, 128]])
                        nc.vector.tensor_copy(
                            dst, acc[64 * g:64 * (g + 1), :])

            def _a_w(grp, ck):
                if grp < 4:
                    return wq_sb[:, ck * 512 + grp * 128:
                                 ck * 512 + (grp + 1) * 128]
                w_t = wk_sb if grp == 4 else wv_sb
                return w_t[:, ck * 128:(ck + 1) * 128]

            def _a_mm128(tb, grp, acc, ck):
                nc.tensor.matmul(
                    acc, _a_w(grp, ck),
                    x_sb[:, ds(tb * 512 + ck * 2048, 512)],
                    start=(ck == 0), stop=(ck == 15))

            def _a_vtp(tb):
                # V_aug build for this tb's 4 key tiles: transpose VT
                # 128-blocks into [tok, vdim] tiles
                for i in range(4):
                    kt = 4 * tb + i
                    vtp = ps.tile([128, 128], BF16, tag="pa", bufs=2,
                                  name=f"vtp{kt}")
                    nc.tensor.transpose(
                        vtp, vt_sb[:, kt * 128:(kt + 1) * 128], idl)
                    # vtp cols 0-63 = g0 vdims -> va tile kt; 64-127 = g1
                    # vdims -> va tile 16+kt (ones block at +64 untouched)
                    dest = bass.AP(tensor=va_sb.tensor,
                                   offset=va_sb.offset + kt * 128,
                                   ap=[va_sb.ap[0], [16 * 128, 2], [1, 64]])
                    src = bass.AP(tensor=vtp.tensor, offset=vtp.offset,
                                  ap=[vtp.ap[0], [64, 2], [1, 64]])
                    nc.vector.tensor_copy(dest, src)

            def phase_a_wide(tb):
                # pre-B: all 6 accumulation groups in flight, ck-major, so
                # the PE tracks the input DMA stream; K=128 full mode (the
                # one tiling-mode switch happens at the A->B boundary)
                tags = ["og", "og", "og", "og", "st", "st"]
                accs = [ps.tile([128, 512], F32, tag=tags[grp],
                                bufs=(og_bufs if grp < 4 else st_bufs),
                                name=f"aw{tb}_{grp}") for grp in range(6)]
                for ck in range(16):
                    for grp in range(6):
                        _a_mm128(tb, grp, accs[grp], ck)
                for grp in range(6):
                    _a_evacuate(tb, grp, accs[grp])
                for i in range(4):
                    kt = 4 * tb + i
                    vtp = ps.tile([128, 128], BF16, tag="og", bufs=og_bufs,
                                  name=f"vtp{kt}")
                    nc.tensor.transpose(
                        vtp, vt_sb[:, kt * 128:(kt + 1) * 128], idl)
                    dest = bass.AP(tensor=va_sb.tensor,
                                   offset=va_sb.offset + kt * 128,
                                   ap=[va_sb.ap[0], [16 * 128, 2], [1, 64]])
                    src2 = bass.AP(tensor=vtp.tensor, offset=vtp.offset,
                                   ap=[vtp.ap[0], [64, 2], [1, 64]])
                    nc.vector.tensor_copy(dest, src2)

            def phase_a_wide(tb):
                # pre-B: all 6 accumulation groups in flight, ck-major, so
                # the PE tracks the input DMA stream; K=128 full mode (the
                # one tiling-mode switch happens at the A->B boundary)
                tags = ["og", "og", "og", "og", "st", "st"]
                accs = [ps.tile([128, 512], F32, tag=tags[grp],
                                bufs=(og_bufs if grp < 4 else st_bufs),
                                name=f"aw{tb}_{grp}") for grp in range(6)]
                for ck in range(16):
                    for grp in range(6):
                        _a_mm128(tb, grp, accs[grp], ck)
                for grp in range(6):
                    _a_evacuate(tb, grp, accs[grp])
                for i in range(4):
                    kt = 4 * tb + i
                    vtp = ps.tile([128, 128], BF16, tag="og", bufs=og_bufs,
                                  name=f"vtp{kt}")
                    nc.tensor.transpose(
                        vtp, vt_sb[:, kt * 128:(kt + 1) * 128], idl)
                    dest = bass.AP(tensor=va_sb.tensor,
                                   offset=va_sb.offset + kt * 128,
                                   ap=[va_sb.ap[0], [16 * 128, 2], [1, 64]])
                    src2 = bass.AP(tensor=vtp.tensor, offset=vtp.offset,
                                   ap=[vtp.ap[0], [64, 2], [1, 64]])
                    nc.vector.tensor_copy(dest, src2)

            def a_chunks(tb):
                """Phase A(tb) as background chunks: 6 accumulation groups
                (16 matmul pairs + merge-evac each) plus the V_aug
                transposes. Emitted one per phase-B batch slot so A's PE
                work fills B's exp-bound slack."""
                def grp_chunk(grp):
                    def run():
                        acc2 = [ps.tile([128, 512], F32, tag="pa", bufs=2,
                                        name=f"pa{tb}_{grp}_{hh}")
                                for hh in range(2)]
                        for ck in range(16):
                            _a_mm64(tb, grp, acc2, ck)
                        tmp = mrg.tile([128, 512], F32, tag="mrg",
                                       name=f"mt{tb}_{grp}")
                        nc.scalar.copy(tmp, acc2[1])
                        nc.vector.tensor_add(
                            qt_sb[:, 0:1], acc2[0], tmp)  # unused path
                    return run
                return ([("A", tb, grp_chunk(g)) for g in range(6)]
                        + [("A", tb, lambda: _a_vtp(tb))])

            def phase_a0():
                # ck-major variant for tb=0: all 6 accumulation groups in
                # flight (B hasn't started; its st/og banks are free), so the
                # PE tracks the input DMA stream instead of stalling on two
                # serialized groups.
                tags = ["pa", "pa", "st", "st", "og", "og"]
                tbufs = {"pa": 2, "st": st_bufs, "og": og_bufs}
                accs = [ps.tile([128, 512], F32, tag=tags[grp],
                                bufs=tbufs[tags[grp]],
                                name=f"pa0_{grp}") for grp in range(6)]
                for ck in range(16):
                    for grp in range(6):
                        _a_mm128(0, grp, accs[grp], ck)
                for grp in range(6):
                    _a_evacuate(tb, grp, accs[grp])
                for i in range(4):
                    vtp = ps.tile([128, 128], BF16, tag="pa", bufs=2,
                                  name=f"vtp0_{i}")
                    nc.tensor.transpose(
                        vtp, vt_sb[:, i * 128:(i + 1) * 128], idl)
                    dest = bass.AP(tensor=va_sb.tensor,
                                   offset=va_sb.offset + i * 128,
                                   ap=[va_sb.ap[0], [16 * 128, 2], [1, 64]])
                    src = bass.AP(tensor=vtp.tensor, offset=vtp.offset,
                                  ap=[vtp.ap[0], [64, 2], [1, 64]])
                    nc.vector.tensor_copy(dest, src)

            def _batches(nkt):
                """Key-tile batches of up to 4, DIAGONAL TILES FIRST: their
                mask multiplies then overlap later tiles' scores/exps
                instead of serializing the block tail (AV accumulation
                order is arbitrary)."""
                order = ([nkt - 2, nkt - 1] + list(range(nkt - 2))
                         if diag_first else list(range(nkt)))
                return [order[i:i + 4] for i in range(0, nkt, 4)]

            carry = [None]   # deferred tail (last AVs + normalize) per (j,hf)

            def flush_b(filler=None):
                if carry[0] is not None:
                    carry[0]()
                    carry[0] = None
                    if filler is not None:
                        filler()

            def phase_b(j, filler=None):
                """Everything in here runs in the PE's 64x128 tiling mode:
                scores are (0,0)/(64,0) row-tile pairs, and AV matmuls split
                their 128-token contraction into two 64-row halves
                accumulating into separate og banks (merged by the DVE in
                the normalize step). No tiling-mode drains in the loop.

                Each (j,hf)'s last AV batch + normalize is deferred and
                emitted inside the NEXT (j,hf)'s first batch, right after
                its first score pair, so the PE never sits at a boundary."""
                NKT = 2 * j + 2
                for hf in range(2):     # rep-halves of the 1024-wide q block
                    og = [ps.tile([128, 512], F32, tag="og", bufs=og_bufs,
                                  name=f"og{j}_{hf}_{g}") for g in range(2)]

                    def av1(pkt, g, p, first, last, og=og):
                        nc.tensor.matmul(
                            og[g],
                            va_sb[:, (g * 16 + pkt) * 128:
                                  (g * 16 + pkt) * 128 + 128],
                            p, start=first, stop=last)

                    def norm(og=og, j=j, hf=hf):
                        if b_no_norm:
                            return
                        # og rows 64-127 hold the denominator replicated
                        # across 64 partitions; recip and the normalize
                        # muls read og directly (one PSUM input each). og
                        # is double-buffered across (j,hf) blocks, so this
                        # never stalls the next block's AV matmuls.
                        for g in range(2):
                            rec = rcp.tile([64, 512], F32, tag="rec",
                                           name=f"r{j}{hf}{g}")
                            nc.vector.reciprocal(rec, og[g][64:128, :])
                            oc = 2 * g + hf
                            for r in range(2):   # rep 2hf, 2hf+1
                                dst = otn_sb if r == 0 else otn2_sb
                                nc.vector.tensor_mul(
                                    dst[0:64, ds(oc * 2048 + j * 256, 256)],
                                    og[g][0:64, r * 256:(r + 1) * 256],
                                    rec[:, r * 256:(r + 1) * 256])

                    def score(kt, g):
                        # K=128 full-mode matmul; the block-diagonal zero
                        # rows of the weight tile select group g from the
                        # stacked Q (no tiling-mode changes anywhere)
                        st = ps.tile([128, 512], F32, tag="st",
                                     bufs=st_bufs, name=f"st{j}{hf}{kt}{g}")
                        nc.tensor.matmul(
                            st,
                            kt_bd[:, (2 * kt + g) * 128:
                                  (2 * kt + g + 1) * 128],
                            qt_sb[:, ds(j * 1024 + hf * 512, 512)],
                            start=True, stop=True)
                        p = ppool.tile([128, 512], BF16, tag="p",
                                       name=f"p{j}{hf}{kt}{g}")
                        if b_exp_dve:
                            nc.vector.tensor_copy(p, st)
                        elif not b_no_exp:
                            nc.scalar.activation(p, st, EXP, scale=0.125)
                        # causal mask: diagonal key tiles only
                        if kt >= 2 * j and not b_no_mask:
                            moff = 256 * (1 + kt - 2 * j)
                            mask_b = bass.AP(
                                tensor=msk_sb.tensor,
                                offset=msk_sb.offset + moff,
                                ap=[msk_sb.ap[0], [0, 2], [1, 256]])
                            eng = nc.gpsimd if mask_gpsimd else nc.vector
                            eng.tensor_mul(p, p, mask_b)
                        return p

                    prev = None
                    seen = 0
                    for batch in _batches(NKT):
                        cur = []
                        for idx, kt in enumerate(batch):
                            for g in range(2):
                                cur.append((kt, g, score(kt, g), seen == 0))
                            seen += 1
                            if idx == 0:
                                # PE work to overlap this batch's exps: the
                                # previous batch's AVs, or at a (j,hf)
                                # boundary the deferred tail of the previous
                                # block
                                if prev is not None:
                                    for pkt, g, p, first in prev:
                                        av1(pkt, g, p, first, False)
                                    if filler is not None:
                                        filler()
                                else:
                                    flush_b(filler)
                        prev = cur

                    def tail(prev=prev, av1=av1, norm=norm):
                        lastg = {}
                        for i, (pkt, g, p, first) in enumerate(prev):
                            lastg[g] = i
                        for i, (pkt, g, p, first) in enumerate(prev):
                            av1(pkt, g, p, first, lastg[g] == i)
                        norm()
                    carry[0] = tail

            def stage_otn(th):
                # odd-rep otn rows (otn2, partitions 0-63) -> otn rows 64-127
                # for this token half
                for oc in range(4):
                    nc.sync.dma_start(
                        out=otn_sb[64:128, ds(oc * 2048 + th * 1024, 1024)],
                        in_=otn2_sb[0:64, ds(oc * 2048 + th * 1024, 1024)])

            def c_block(th, oc):
                # post-B: K=128 full mode, yt rotating through both freed
                # bank sets so PE never waits on evacuation
                for qh in range(2):
                    tag = "st" if (oc * 2 + qh) % 2 == 0 else "og"
                    yt = ps.tile([128, 512], F32, tag=tag,
                                 bufs=(st_bufs if tag == "st" else og_bufs),
                                 name=f"yt{th}_{oc}_{qh}")
                    for ic in range(4):
                        nc.tensor.matmul(
                            yt,
                            wo_sb[:, ic * 2048 + oc * 128:
                                  ic * 2048 + (oc + 1) * 128],
                            otn_sb[:, ds(ic * 2048 + th * 1024 + qh * 512,
                                         512)],
                            start=(ic == 0), stop=(ic == 3))
                    yt_sb = ytp.tile([128, 512], BF16, tag="ytsb",
                                     name=f"ytsb{th}_{oc}_{qh}")
                    nc.vector.tensor_copy(yt_sb, yt)
                    nc.sync.dma_start(
                        out=YT[oc * 128:(oc + 1) * 128,
                               ds(th * 1024 + qh * 512, 512)],
                        in_=yt_sb)

            def body(_rep):
                # ---------------- input DMA ----------------
                # host pre-arranges every input into its SBUF layout; x/wq/
                # wk/wv stream in 4 ck-groups so phase A can start on group 0
                # while later groups are in flight; wo (phase C only) last.
                nc.sync.dma_start(out=msk_sb, in_=MSK4)
                for cg in range(4):
                    nc.sync.dma_start(out=x_sb[:, ds(cg * 8192, 8192)],
                                      in_=xT[:, ds(cg * 8192, 8192)])
                    nc.sync.dma_start(out=wq_sb[:, ds(cg * 2048, 2048)],
                                      in_=WqT[:, ds(cg * 2048, 2048)])
                    nc.sync.dma_start(out=wk_sb[:, ds(cg * 512, 512)],
                                      in_=WkT[:, ds(cg * 512, 512)])
                    nc.sync.dma_start(out=wv_sb[:, ds(cg * 512, 512)],
                                      in_=WvT[:, ds(cg * 512, 512)])
                nc.sync.dma_start(out=wo_sb, in_=WoT)

                # ---------------- A then B then C (serial phases; B is
                # internally pipelined and switch-free) -------------------
                A, Bp, C = ("A" in phases), ("B" in phases), ("C" in phases)
                if Bp and not A:   # diagnostic builds: satisfy the tracker
                    for t in (qt_sb, kt_bd, vt_sb, va_sb):
                        nc.vector.memset(t, 0.01)
                    if b_no_norm:
                        nc.vector.memset(otn_sb, 0.01)
                        nc.vector.memset(otn2_sb, 0.01)
                if A:
                    for tb in range(4):
                        phase_a_wide(tb)
                if Bp:
                    for j in range(4):
                        phase_b(j)
                    flush_b()
                    stage_otn(0)   # th0 staging DMA overlaps B(4..7)
                    for j in range(4, 8):
                        phase_b(j)
                    flush_b()
                elif C:
                    stage_otn(0)
                if C:
                    stage_otn(1)
                    for oc in range(16):
                        c_block(0, oc)
                    for oc in range(16):
                        c_block(1, oc)

            loop(loop_n, body)

    nc.compile()
    return nc


def _get_nc():
    if "nc" not in _CACHE:
        _CACHE["nc"] = _build_nc()
    return _CACHE["nc"]


# --------------------------------------------------------------------------
# host wrapper
# --------------------------------------------------------------------------
def _bf16(a):
    import ml_dtypes
    return np.ascontiguousarray(np.asarray(a).astype(ml_dtypes.bfloat16))


def _make_mask() -> np.ndarray:
    """4 multiplicative 0/1 blocks of [128, 256] (broadcast over reps):
    block 0: all-pass; 1: diag kt==2j; 2: diag kt==2j+1; 3: all-blocked."""
    ki = np.arange(128)[:, None]
    qi = np.arange(256)[None, :]
    o = np.ones((128, 256), np.float32)
    m0 = np.where(ki <= qi, 1.0, 0.0).astype(np.float32)
    m1 = np.where(128 + ki <= qi, 1.0, 0.0).astype(np.float32)
    mf = np.zeros((128, 256), np.float32)
    return np.concatenate([o, m0, m1, mf], axis=1)  # [128, 1024]


def _core_inputs(x, Wq, Wk, Wv, Wo, c, mask):
    b, hb = c // 4, c % 4
    xT_c = np.ascontiguousarray(x[b].T)
    # interleave q heads: chunk qc = [g0 rep qc (64) | g1 rep qc (64)]
    g0, g1 = 2 * hb, 2 * hb + 1
    cols = []
    for qc in range(NREP):
        cols.append(Wq[g0 * 256 + qc * 64: g0 * 256 + (qc + 1) * 64])
        cols.append(Wq[g1 * 256 + qc * 64: g1 * 256 + (qc + 1) * 64])
    WqT_c = np.ascontiguousarray(np.concatenate(cols, axis=0).T)
    WkT_c = np.ascontiguousarray(Wk[128 * hb:128 * (hb + 1)].T)
    WvT_c = np.ascontiguousarray(Wv[128 * hb:128 * (hb + 1)].T)
    WoT_c = np.ascontiguousarray(Wo[:, 512 * hb:512 * (hb + 1)].T)
    def _sb(a, nchunk):    # [nchunk*128, w] -> [128, nchunk*w] (ck-major cols)
        n = a.shape[0] // 128
        assert n == nchunk
        return a.reshape(n, 128, a.shape[1]).transpose(1, 0, 2).reshape(
            128, n * a.shape[1])
    return {"xT": _bf16(_sb(xT_c, 16)), "WqT": _bf16(_sb(WqT_c, 16)),
            "WkT": _bf16(_sb(WkT_c, 16)), "WvT": _bf16(_sb(WvT_c, 16)),
            "WoT": _bf16(_sb(WoT_c, 4)), "MSK4": _bf16(mask)}


def kernel(x, Wq, Wk, Wv, Wo, _trace=False, _trace_kwargs=None):
    from concourse import bass_utils

    x = np.asarray(x, dtype=np.float32)
    Wq = np.asarray(Wq, dtype=np.float32)
    Wk = np.asarray(Wk, dtype=np.float32)
    Wv = np.asarray(Wv, dtype=np.float32)
    Wo = np.asarray(Wo, dtype=np.float32)

    nc = _get_nc()
    mask = _make_mask()
    in_maps = [_core_inputs(x, Wq, Wk, Wv, Wo, c, mask) for c in range(8)]

    res = None
    last_exc = None
    for _attempt in range(3):
        try:
            res = bass_utils.run_bass_kernel_spmd(
                nc, in_maps, core_ids=list(range(8)),
                trace=_trace, **(_trace_kwargs or {}))
            break
        except Exception as e:  # transient device wedges happen; retry
            last_exc = e
    if res is None:
        raise last_exc

    Y = np.zeros((B, T, D_MODEL), dtype=np.float32)
    for c in range(8):
        Y[c // 4] += res.results[c]["YT"].T.astype(np.float32)
    if _trace:
        _CACHE["last_result"] = res
    return Y
